# revision 1
# baseline (speedup 1.0000x reference)
"""MBart MoE decoder layer on 8 trn2 NeuronCores.

Sharding: 8 cores = 8 (sequence, expert-slot) pairs. Core c handles
sequence b=c//2, expert slot j=c%2 (each sequence is lang-routed to at
most 2 distinct experts; routing is computed on the host from `langs`).
Each core computes the full attention path for its sequence (replicated
across the pair) and one expert FFN over all 256 tokens; the host sums
the pair's partial outputs (expert-sharded combine) and transposes back
to token-major. Expert weights are gathered per-core on the host, so a
core only receives the one expert it needs.

On-device layout is feature-major [D, tokens]: projections take weights
as lhsT (feature-major out) or activations as lhsT (token-major out), so
no activation transposes are needed anywhere. LN gains/biases are folded
into the downstream weights on the host; softmax uses transposed scores
[keys, queries] with the attention mask added via an identity-matmul
into PSUM (host classifies each 128x128 mask block as zero / add / skip,
so causal dead blocks are never computed) and denominators accumulated
via a ones-matmul, then broadcast over partitions with a rank-1 matmul
for one full-lane reciprocal per head pair.
"""

import os
import sys
from contextlib import ExitStack

for _p in ("/opt/trn_rl_repo",):
    if _p not in sys.path:
        sys.path.append(_p)

import numpy as np
import ml_dtypes

import concourse.bass as bass
import concourse.tile as tile
import concourse.mybir as mybir
from concourse import bacc, bass_utils

B, S, SK = 4, 256, 512
D, NH, NKV, HD = 1024, 16, 4, 64
DE, NE = 4096, 8
LN_EPS = 1e-5
REP = NH // NKV
DC = D // 128    # 8 feature chunks
FC = DE // 128   # 32 ffn chunks
SC = S // 128    # 2 self-attn key chunks
KC = SK // 128   # 4 cross-attn key chunks
QC = S // 128    # 2 query halves
KVW = NKV * HD   # 256
GRP = 4          # ffn chunks per MoE weight group
NGRP = FC // GRP

MODE = os.environ.get("KERNEL_MM_DTYPE", "bf16")  # "bf16" | "f32r" | "f32"

_CACHE: dict = {}
_TRACE_DIR = None   # set by test harness for profiling runs
_LAST_EXEC_NS = None

# packed attention-weight column layout: qw | kw(dup) | vw
W_Q, W_K, W_V = 0, D, D + 2 * KVW
WPACK = D + 2 * KVW + KVW  # 1792

# packed per-partition bias column layout
_BIAS_COLS = {}
_off = 0
for _n, _w in [("qb", DC), ("kb", 4), ("vb", KVW), ("ob", DC),
               ("q2b", DC), ("k2b", 4), ("v2b", KVW), ("o2b", DC),
               ("b1", FC), ("b3", FC), ("c", 1)]:
    _BIAS_COLS[_n] = (_off, _w)
    _off += _w
BIAS_W = _off


def _build(mode, sa_cls, ca_cls):
    """sa_cls/ca_cls: block classes per (kc, qhalf): 0=no-mask, 1=mask-add,
    2=fully-masked(skip)."""
    st = {"bf16": mybir.dt.bfloat16, "f32r": mybir.dt.float32r,
          "f32": mybir.dt.float32}[mode]
    f32 = mybir.dt.float32
    same_st = mode == "f32"
    A = mybir.ActivationFunctionType
    OP = mybir.AluOpType

    nc = bacc.Bacc("TRN2", target_bir_lowering=False, debug=False, num_devices=8)

    def mm(psum, lhsT, rhs, start, stop):
        nc.tensor.matmul(psum, lhsT, rhs, start=start, stop=stop)

    di = {}

    def din(name, shape, dtype=None):
        di[name] = nc.dram_tensor(name, list(shape), dtype or st, kind="ExternalInput")
        return di[name]

    din("xT", (D, S), f32)
    if not same_st:
        din("xT_st", (D, S))
    din("encT", (D, SK))
    need_samask = any(c == 1 for c in sa_cls)
    need_camask = any(c == 1 for c in ca_cls)
    if need_samask:
        din("maskT", (S, S))
    if need_camask:
        din("encmaskT", (SK, S))
    din("id128", (128, 128))
    din("ones_col", (128, 1))
    din("ones_row", (1, 128))
    din("wqkv", (D, WPACK))
    din("wca", (D, WPACK))
    din("ow", (D, D))
    din("o2w", (D, D))
    din("biases", (128, BIAS_W), f32)
    din("w13", (D, 2 * DE))   # interleaved per group: [w1 g-cols | w3 g-cols]
    din("w2", (DE, D))
    out_res = nc.dram_tensor("out_res", [D, S], f32, kind="ExternalOutput")
    out_ffn = nc.dram_tensor("out_ffn", [S, D], f32, kind="ExternalOutput")

    with tile.TileContext(nc) as tc, ExitStack() as ctx:
        cp = ctx.enter_context(tc.tile_pool(name="consts", bufs=1))
        pers = ctx.enter_context(tc.tile_pool(name="pers", bufs=1))

        ones128 = cp.tile([128, 1], st, tag="ones128", name="ones128")
        nc.sync.dma_start(ones128[:], di["ones_col"].ap())
        ones1r = cp.tile([1, 128], st, tag="ones1r", name="ones1r")
        nc.sync.dma_start(ones1r[:], di["ones_row"].ap())
        eps_t = cp.tile([128, 1], f32, tag="eps_t", name="eps_t")
        nc.vector.memset(eps_t, LN_EPS)
        id128 = cp.tile([128, 128], st, tag="id128", name="id128")
        nc.sync.dma_start(id128[:], di["id128"].ap())
        maskT = encmaskT = None
        if need_samask:
            maskT = cp.tile([128, SC, S], st, tag="maskT", name="maskT")
            for kc in range(SC):
                nc.sync.dma_start(maskT[:, kc, :],
                                  di["maskT"].ap()[kc * 128:(kc + 1) * 128, :])
        if need_camask:
            encmaskT = cp.tile([128, KC, S], st, tag="encmaskT", name="encmaskT")
            for kc in range(KC):
                nc.sync.dma_start(encmaskT[:, kc, :],
                                  di["encmaskT"].ap()[kc * 128:(kc + 1) * 128, :])

        bias_t = cp.tile([128, BIAS_W], f32, tag="bias_t", name="bias_t")
        nc.sync.dma_start(bias_t[:], di["biases"].ap())

        def bias(nm):
            off, w = _BIAS_COLS[nm]
            return bias_t[:, off:off + w]

        def load_chunks(dram, nchunk, width, tag, pool, dtype=st):
            ts = []
            for k in range(nchunk):
                t = pool.tile([128, width], dtype, tag=f"{tag}{k}", name=f"{tag}{k}")
                nc.sync.dma_start(t[:], dram.ap()[k * 128:(k + 1) * 128, :])
                ts.append(t)
            return ts

        def layernorm(src_f32, src_st, out_tag, pool):
            """src: DC chunks [128,S] f32 (+st copies). Returns DC normalized
            chunks [128,S] st (gain/bias folded downstream by host)."""
            with tc.tile_pool(name=f"{out_tag}_lt", bufs=2) as lp, \
                 tc.tile_pool(name=f"{out_tag}_lp", bufs=1, space="PSUM") as sp, \
                 tc.tile_pool(name=f"{out_tag}_lb", bufs=1, space="PSUM") as bp:
                sum_ps = sp.tile([1, S], f32, tag="lnsum", name="lnsum")
                sq_ps = sp.tile([1, S], f32, tag="lnsq", name="lnsq")
                for k in range(DC):
                    sq = lp.tile([128, S], st, tag="lnsqt", name="lnsqt")
                    nc.vector.tensor_tensor(sq[:], src_f32[k][:], src_f32[k][:],
                                            OP.mult)
                    mm(sum_ps[:], ones128[:], src_st[k][:], k == 0, k == DC - 1)
                    mm(sq_ps[:], ones128[:], sq[:], k == 0, k == DC - 1)
                s_sb = lp.tile([1, S], st, tag="ln_ssb", name="ln_ssb")
                nc.vector.tensor_single_scalar(s_sb[:], sum_ps[:], 1.0 / D, OP.mult)
                q_sb = lp.tile([1, S], st, tag="ln_qsb", name="ln_qsb")
                nc.vector.tensor_single_scalar(q_sb[:], sq_ps[:], 1.0 / D, OP.mult)
                s_bc = bp.tile([128, S], f32, tag="ln_sbc", name="ln_sbc")
                q_bc = bp.tile([128, S], f32, tag="ln_qbc", name="ln_qbc")
                mm(s_bc[:], ones1r[:], s_sb[:], True, True)   # mean, bcast
                mm(q_bc[:], ones1r[:], q_sb[:], True, True)   # E[x^2], bcast
                # full-lane stats math on [128,S]
                s_sbuf = lp.tile([128, S], f32, tag="ln_ssbuf", name="ln_ssbuf")
                nc.vector.tensor_copy(s_sbuf[:], s_bc[:])
                var = lp.tile([128, S], f32, tag="ln_var", name="ln_var")
                nc.vector.scalar_tensor_tensor(var[:], s_bc[:], 0.0, s_sbuf[:],
                                               OP.bypass, OP.mult)
                nc.vector.tensor_sub(var[:], q_bc[:], var[:])
                v_t = lp.tile([128, S], f32, tag="ln_vt", name="ln_vt")
                nc.scalar.activation(v_t[:], var[:], A.Abs_reciprocal_sqrt,
                                     bias=eps_t[:])
                u_t = lp.tile([128, S], f32, tag="ln_ut", name="ln_ut")
                nc.vector.tensor_tensor(u_t[:], s_sbuf[:], v_t[:], OP.mult)
                outs = []
                for k in range(DC):
                    o = pool.tile([128, S], st, tag=f"{out_tag}{k}",
                                  name=f"{out_tag}{k}")
                    nc.vector.tensor_tensor(o[:], src_f32[k][:], v_t[:], OP.mult)
                    nc.vector.tensor_sub(o[:], o[:], u_t[:])
                    outs.append(o)
                return outs

        def cast_st(src, tag, pool):
            if same_st:
                return src
            outs = []
            for k, t in enumerate(src):
                o = pool.tile([128, t.shape[-1]], st, tag=f"{tag}{k}",
                              name=f"{tag}{k}")
                nc.vector.tensor_copy(o[:], t[:])
                outs.append(o)
            return outs

        def project_fm(w_slices, rhs_chunks, nout, bias_ap, out_tag, pool,
                       extra=None, out_dt=None, width=None):
            """out^T[dout_chunk] = sum_k w_slices[k][:, m*128:...].T @ rhs[k]."""
            W = width or S
            outs = []
            with tc.tile_pool(name=f"{out_tag}_ps", bufs=3, space="PSUM") as pp:
                for mI in range(nout):
                    ps = pp.tile([128, W], f32, tag="proj", name="proj")
                    for k in range(DC):
                        mm(ps[:], w_slices[k][:, mI * 128:(mI + 1) * 128],
                           rhs_chunks[k][:], k == 0, k == DC - 1)
                    o = pool.tile([128, W], out_dt or st, tag=f"{out_tag}{mI}",
                                  name=f"{out_tag}{mI}")
                    if extra is not None:
                        extra(mI, ps, o)
                    elif bias_ap is not None:
                        nc.vector.tensor_scalar(o[:], ps[:],
                                                bias_ap[:, mI:mI + 1], None,
                                                OP.add)
                    else:
                        nc.vector.tensor_copy(o[:], ps[:])
                    outs.append(o)
            return outs

        def project_tm(act_chunks, w_slices, ntok, bias_bcast, out_tag, pool):
            """token-major V with a ones column appended per kv head:
            out[tok_chunk] = [V_kv | 1] blocks of 65 columns."""
            outs = []
            with tc.tile_pool(name=f"{out_tag}_ps", bufs=3, space="PSUM") as pp:
                for t in range(ntok):
                    ps = pp.tile([128, KVW], f32, tag="projtm", name="projtm")
                    for k in range(DC):
                        mm(ps[:], act_chunks[k][:, t * 128:(t + 1) * 128],
                           w_slices[k][:], k == 0, k == DC - 1)
                    o = pool.tile([128, NKV, HD + 1], st, tag=f"{out_tag}{t}",
                                  name=f"{out_tag}{t}")
                    nc.vector.tensor_add(
                        o[:, :, 0:HD],
                        ps[:].rearrange("p (kv d) -> p kv d", kv=NKV),
                        bias_bcast[:].rearrange("p (kv d) -> p kv d", kv=NKV))
                    for kv in range(NKV):
                        nc.vector.tensor_copy(o[:, kv, HD:HD + 1], ones128[:])
                    outs.append(o)
            return outs

        def attend(qT, kT, vtm, n_kc, mask_tile, cls, out_tag, pool):
            """Transposed-score attention. cls[kc*QC + qh] in {0,1,2}.
            vtm blocks are [V_kv | ones] of 65 cols, so each O' matmul also
            accumulates the softmax denominator into row 64."""
            outs = []
            # per kc: active query range (contiguous union of non-skip halves)
            qr = []
            for kc in range(n_kc):
                act = [qh for qh in range(QC) if cls[kc * QC + qh] != 2]
                assert act and act == list(range(act[0], act[-1] + 1))
                qr.append((act[0] * 128, (act[-1] + 1) * 128))
            with tc.tile_pool(name=f"{out_tag}_sp", bufs=3, space="PSUM") as stp, \
                 tc.tile_pool(name=f"{out_tag}_op", bufs=2, space="PSUM") as opp, \
                 tc.tile_pool(name=f"{out_tag}_bp", bufs=1, space="PSUM") as bpp, \
                 tc.tile_pool(name=f"{out_tag}_et", bufs=6) as epool, \
                 tc.tile_pool(name=f"{out_tag}_dt", bufs=3) as dpool:
                for c in range(DC):
                    o_ps_h = [opp.tile([65, S], f32, tag=f"oph{hh}",
                                       name=f"oph{hh}") for hh in range(2)]
                    kv = (2 * c) // REP      # same kv head for both of the pair
                    for kc in range(n_kc):
                        q0, q1 = qr[kc]
                        adds = [q for q in range(QC) if cls[kc * QC + q] == 1]
                        st_h = []
                        e_h = []
                        for hh in range(2):
                            qh_ap = qT[c][hh * 64:(hh + 1) * 64, :]
                            kh = kT[kv][hh * 64:(hh + 1) * 64, :]
                            st_ps = stp.tile([128, S], f32, tag="st",
                                             name="st")
                            mm(st_ps[:, q0:q1], kh[:, kc * 128:(kc + 1) * 128],
                               qh_ap[:, q0:q1], True, not adds)
                            st_h.append(st_ps)
                        for hh in range(2):
                            for i, q in enumerate(adds):
                                mm(st_h[hh][:, q * 128:(q + 1) * 128], id128[:],
                                   mask_tile[:, kc, q * 128:(q + 1) * 128],
                                   False, i == len(adds) - 1)
                        for hh in range(2):
                            e = epool.tile([128, S], st, tag="e", name="e")
                            nc.scalar.activation(e[:, q0:q1],
                                                 st_h[hh][:, q0:q1], A.Exp)
                            e_h.append(e)
                        for hh in range(2):
                            mm(o_ps_h[hh][:, q0:q1],
                               vtm[kc][:, kv, :], e_h[hh][:, q0:q1],
                               kc == 0, kc == n_kc - 1)
                    den_pair = dpool.tile([1, 2 * S], st, tag="den_pair",
                                          name="den_pair")
                    for hh in range(2):
                        nc.vector.tensor_copy(den_pair[:, hh * S:(hh + 1) * S],
                                              o_ps_h[hh][64:65, :])
                    r_ps = bpp.tile([128, 2 * S], f32, tag="rbc", name="rbc")
                    mm(r_ps[:], ones1r[:], den_pair[:], True, True)
                    # 1/x as (1/sqrt(x))^2 on ACT; avoids the slow DVE recip
                    rsq = dpool.tile([128, 2 * S], f32, tag="rsq", name="rsq")
                    nc.scalar.activation(rsq[:], r_ps[:], A.Abs_reciprocal_sqrt)
                    rbi = dpool.tile([128, 2 * S], f32, tag="rbi", name="rbi")
                    nc.vector.tensor_tensor(rbi[:], rsq[:], rsq[:], OP.mult)
                    o = pool.tile([128, S], st, tag=f"{out_tag}{c}",
                                  name=f"{out_tag}{c}")
                    for hh in range(2):
                        nc.vector.tensor_tensor(
                            o[hh * 64:(hh + 1) * 64, :], o_ps_h[hh][0:64, :],
                            rbi[hh * 64:(hh + 1) * 64, hh * S:(hh + 1) * S],
                            OP.mult)
                    outs.append(o)
            return outs

        h1 = [pers.tile([128, S], f32, tag=f"h1T{k}", name=f"h1T{k}")
              for k in range(DC)]
        h2 = [pers.tile([128, S], f32, tag=f"h2T{k}", name=f"h2T{k}")
              for k in range(DC)]

        # ---------------- self attention ----------------
        with tc.tile_pool(name="sa_acts", bufs=1) as sa:
            xT = load_chunks(di["xT"], DC, S, "xT", sa, f32)
            xT_st = xT if same_st else load_chunks(di["xT_st"], DC, S, "xTs", sa)
            n1 = layernorm(xT, xT_st, "n1T", sa)
            with tc.tile_pool(name="wqkvp", bufs=1) as wp:
                wt = load_chunks(di["wqkv"], DC, WPACK, "wqkv", wp)
                qT = project_fm([t[:, W_Q:W_Q + D] for t in wt], n1, DC,
                                bias("qb"), "qT", sa)
                kT = project_fm([t[:, W_K:W_K + 2 * KVW] for t in wt], n1, 4,
                                bias("kb"), "kT", sa)
                v_tm = project_tm(n1, [t[:, W_V:W_V + KVW] for t in wt], SC,
                                  bias("vb"), "v_tm", sa)
            with tc.tile_pool(name="wop", bufs=1) as wp:
                ow_t = load_chunks(di["ow"], DC, D, "ow", wp)
                sa_out = attend(qT, kT, v_tm, SC, maskT, sa_cls, "saT", sa)

                def o_epil(mI, ps, o):
                    nc.vector.scalar_tensor_tensor(o[:], ps[:],
                                                   bias("ob")[:, mI:mI + 1],
                                                   xT[mI][:], OP.add, OP.add)
                project_fm(ow_t, sa_out, DC, None, "h1w", _FixedPool(h1),
                           extra=o_epil, out_dt=f32)

        # ---------------- cross attention ----------------
        with tc.tile_pool(name="ca_acts", bufs=1) as ca:
            encT = load_chunks(di["encT"], DC, SK, "encT", ca)
            h1_st = cast_st(h1, "h1s", ca)
            with tc.tile_pool(name="wcap", bufs=1) as wp:
                wt = load_chunks(di["wca"], DC, WPACK, "wca", wp)
                k2T = project_fm([t[:, W_K:W_K + 2 * KVW] for t in wt], encT, 4,
                                 bias("k2b"), "k2T", ca, width=SK)
                v2_tm = project_tm(encT, [t[:, W_V:W_V + KVW] for t in wt], KC,
                                   bias("v2b"), "v2_tm", ca)
                n2 = layernorm(h1, h1_st, "n2T", ca)
                q2T = project_fm([t[:, W_Q:W_Q + D] for t in wt], n2, DC,
                                 bias("q2b"), "q2T", ca)
            with tc.tile_pool(name="wo2p", bufs=1) as wp:
                o2w_t = load_chunks(di["o2w"], DC, D, "o2w", wp)
                ca_out = attend(q2T, k2T, v2_tm, KC, encmaskT, ca_cls, "caT", ca)

                def o2_epil(mI, ps, o):
                    nc.vector.scalar_tensor_tensor(o[:], ps[:],
                                                   bias("o2b")[:, mI:mI + 1],
                                                   h1[mI][:], OP.add, OP.add)
                project_fm(o2w_t, ca_out, DC, None, "h2w", _FixedPool(h2),
                           extra=o2_epil, out_dt=f32)

        # residual output (host: out_b = res.T + ffn_j0 + ffn_j1)
        for k in range(DC):
            nc.sync.dma_start(out_res.ap()[k * 128:(k + 1) * 128, :], h2[k][:])

        # ---------------- MoE expert ----------------
        with tc.tile_pool(name="moe_acts", bufs=1) as mo:
            h2_st = cast_st(h2, "h2s", mo)
            n3 = layernorm(h2, h2_st, "n3T", mo)

            mT = [mo.tile([128, S], st, tag=f"mT{m}", name=f"mT{m}")
                  for m in range(FC)]
            with tc.tile_pool(name="w13p", bufs=2) as wp, \
                 tc.tile_pool(name="gh_ps", bufs=3, space="PSUM") as gp, \
                 tc.tile_pool(name="gelu_t", bufs=3) as gt:
                gw = GRP * 128
                for g in range(NGRP):
                    wg = []
                    for k in range(DC):
                        t = wp.tile([128, 2 * gw], st, tag=f"w13g{k}",
                                    name=f"w13g{k}")
                        nc.sync.dma_start(t[:], di["w13"].ap()[
                            k * 128:(k + 1) * 128, g * 2 * gw:(g + 1) * 2 * gw])
                        wg.append(t)
                    for mi in range(GRP):
                        mI = g * GRP + mi
                        g_ps = gp.tile([128, S], f32, tag="g_ps", name="g_ps")
                        h_ps = gp.tile([128, S], f32, tag="h_ps", name="h_ps")
                        for k in range(DC):
                            mm(g_ps[:], wg[k][:, mi * 128:(mi + 1) * 128],
                               n3[k][:], k == 0, k == DC - 1)
                        for k in range(DC):
                            mm(h_ps[:], wg[k][:, gw + mi * 128:gw + (mi + 1) * 128],
                               n3[k][:], k == 0, k == DC - 1)
                        ge = gt.tile([128, S], f32, tag="ge", name="ge")
                        nc.scalar.activation(ge[:], g_ps[:], A.Gelu,
                                             bias=bias("b1")[:, mI:mI + 1])
                        nc.vector.scalar_tensor_tensor(mT[mI][:], h_ps[:],
                                                       bias("b3")[:, mI:mI + 1],
                                                       ge[:], OP.add, OP.mult)

            # down-proj, token-major out: y[t,n] = sum_f M^T[f,t].T @ w2[f,n]
            with tc.tile_pool(name="w2p", bufs=3) as wp, \
                 tc.tile_pool(name="y_ps", bufs=1, space="PSUM") as yp, \
                 tc.tile_pool(name="outp", bufs=3) as op_:
                y_ps = [[yp.tile([128, 512], f32, tag=f"y{t}{n}", name=f"y{t}{n}")
                         for n in range(2)] for t in range(QC)]
                for k2 in range(FC):
                    w2t = wp.tile([128, D], st, tag="w2t", name="w2t")
                    nc.sync.dma_start(w2t[:], di["w2"].ap()[k2 * 128:(k2 + 1) * 128, :])
                    for t in range(QC):
                        for n in range(2):
                            mm(y_ps[t][n][:], mT[k2][:, t * 128:(t + 1) * 128],
                               w2t[:, n * 512:(n + 1) * 512],
                               k2 == 0, k2 == FC - 1)
                for t in range(QC):
                    for n in range(2):
                        o = op_.tile([128, 512], f32, tag="o_out", name="o_out")
                        nc.vector.tensor_scalar_mul(o[:], y_ps[t][n][:],
                                                    bias("c")[:, 0:1])
                        nc.sync.dma_start(
                            out_ffn.ap()[t * 128:(t + 1) * 128,
                                         n * 512:(n + 1) * 512], o[:])

    nc.compile()
    return nc


class _FixedPool:
    """Adapter letting project_fm write into pre-allocated tiles."""

    def __init__(self, tiles):
        self._tiles = list(tiles)
        self._i = 0

    def tile(self, shape, dtype, tag=None, name=None):
        t = self._tiles[self._i]
        self._i += 1
        return t


def _routing(langs):
    """Per-sequence expert slots [(expert_idx, coef) x2], matching the
    reference: coef[e,b] = any(langs[b]==4+e) * (1/count(langs[b]>3))."""
    langs = np.asarray(langs)
    slots = []
    for b in range(langs.shape[0]):
        row = [int(v) for v in langs[b]]
        cnt = sum(1 for v in row if v > 3)
        rw = 1.0 if cnt == 0 else 1.0 / cnt
        seen = []
        for v in row:
            if v > 3 and 0 <= v - 4 < NE and (v - 4) not in seen:
                seen.append(v - 4)
        sl = [(e, rw) for e in seen]
        while len(sl) < 2:
            sl.append((0, 0.0))
        slots.append(sl[:2])
    return slots


def _mask_classes(maskT, n_kc):
    """Classify each [128 keys x 128 queries] block of a transposed mask:
    0 all-zero (no add), 1 general (add), 2 fully masked (skip compute).
    Keeps at least one active key block per query and contiguous active
    ranges per key chunk."""
    cls = []
    for kc in range(n_kc):
        for qh in range(QC):
            blk = maskT[kc * 128:(kc + 1) * 128, qh * 128:(qh + 1) * 128]
            if np.all(blk == 0):
                cls.append(0)
            elif np.all(blk <= -1e8):
                cls.append(2)
            else:
                cls.append(1)
    for qh in range(QC):
        if all(cls[kc * QC + qh] == 2 for kc in range(n_kc)):
            for kc in range(n_kc):
                cls[kc * QC + qh] = 1
    for kc in range(n_kc):
        act = [q for q in range(QC) if cls[kc * QC + q] != 2]
        if not act or act != list(range(act[0], act[-1] + 1)):
            for q in range(QC):
                if cls[kc * QC + q] == 2:
                    cls[kc * QC + q] = 1
    return tuple(cls)


def kernel(**inputs):
    mode = MODE
    np_dt = ml_dtypes.bfloat16 if mode == "bf16" else np.float32
    f32 = np.float32

    inp = {k: np.asarray(v) for k, v in inputs.items()}
    x = inp["hidden_states"].astype(f32)
    enc = inp["encoder_hidden_states"].astype(f32)
    mask = inp["attention_mask"].astype(f32)
    encmask = inp["encoder_attention_mask"].astype(f32)
    g1, b1 = inp["ln1_g"].astype(f32), inp["ln1_b"].astype(f32)
    g2, b2 = inp["ln2_g"].astype(f32), inp["ln2_b"].astype(f32)
    g3, b3 = inp["ln3_g"].astype(f32), inp["ln3_b"].astype(f32)

    def dup_kv(w):
        return np.concatenate([np.tile(w[:, 64 * j:64 * (j + 1)], (1, 2))
                               for j in range(NKV)], axis=1)

    def dup_kv_b(v):
        return np.concatenate([np.tile(v[64 * j:64 * (j + 1)], 2)
                               for j in range(NKV)])

    sc = HD ** -0.5
    qw_f = g1[:, None] * inp["sa_q_w"] * sc
    qb_f = (b1 @ inp["sa_q_w"] + inp["sa_q_b"]) * sc
    kw_f = dup_kv(g1[:, None] * inp["sa_k_w"])
    kb_f = dup_kv_b(b1 @ inp["sa_k_w"] + inp["sa_k_b"])
    vw_f = g1[:, None] * inp["sa_v_w"]
    vb_f = b1 @ inp["sa_v_w"] + inp["sa_v_b"]
    q2w_f = g2[:, None] * inp["ca_q_w"] * sc
    q2b_f = (b2 @ inp["ca_q_w"] + inp["ca_q_b"]) * sc
    k2w_f = dup_kv(inp["ca_k_w"])
    k2b_f = dup_kv_b(inp["ca_k_b"])
    w1_f = inp["moe_w1"] * g3[None, :, None]
    b1_f = np.einsum("d,edf->ef", b3, inp["moe_w1"]).astype(f32)
    w3_f = inp["moe_w3"] * g3[None, :, None]
    b3_f = np.einsum("d,edf->ef", b3, inp["moe_w3"]).astype(f32)

    maskT0 = np.ascontiguousarray(mask[:, 0].transpose(0, 2, 1))     # [B,S,S]
    encmaskT0 = np.ascontiguousarray(encmask[:, 0].transpose(0, 2, 1))
    sa_cls = _mask_classes(maskT0[0], SC)
    ca_cls = _mask_classes(encmaskT0[0], KC)
    for b in range(1, B):
        if _mask_classes(maskT0[b], SC) != sa_cls or \
           _mask_classes(encmaskT0[b], KC) != ca_cls:
            sa_cls = tuple(1 for _ in range(SC * QC))
            ca_cls = tuple(1 for _ in range(KC * QC))
            break

    key = (mode, sa_cls, ca_cls)
    if key not in _CACHE:
        _CACHE[key] = _build(mode, sa_cls, ca_cls)
    nc = _CACHE[key]

    def col128(v):
        return np.asarray(v, f32).reshape(-1, 128).T

    slots = _routing(inp["langs"])
    wqkv = np.concatenate([qw_f, kw_f, vw_f], axis=1).astype(np_dt)
    wca = np.concatenate([q2w_f, k2w_f, inp["ca_v_w"]], axis=1).astype(np_dt)

    bias_common = np.zeros((128, BIAS_W), f32)
    for nm, v in [("qb", col128(qb_f)), ("kb", col128(kb_f)),
                  ("vb", np.broadcast_to(vb_f.astype(f32), (128, KVW))),
                  ("ob", col128(inp["sa_o_b"])),
                  ("q2b", col128(q2b_f)), ("k2b", col128(k2b_f)),
                  ("v2b", np.broadcast_to(inp["ca_v_b"].astype(f32), (128, KVW))),
                  ("o2b", col128(inp["ca_o_b"]))]:
        off, w = _BIAS_COLS[nm]
        bias_common[:, off:off + w] = v

    in_maps = []
    for c in range(8):
        b, j = c // 2, c % 2
        e, coef = slots[b][j]
        xT = np.ascontiguousarray(x[b].T)
        # interleave w1/w3 by group: [w1 grp g | w3 grp g] blocks of 512 cols
        gw = GRP * 128
        w13 = np.empty((D, 2 * DE), f32)
        for g in range(NGRP):
            w13[:, g * 2 * gw:g * 2 * gw + gw] = w1_f[e][:, g * gw:(g + 1) * gw]
            w13[:, g * 2 * gw + gw:(g + 1) * 2 * gw] = w3_f[e][:, g * gw:(g + 1) * gw]
        bt = bias_common.copy()
        for nm, v in [("b1", col128(b1_f[e])), ("b3", col128(b3_f[e]))]:
            off, w = _BIAS_COLS[nm]
            bt[:, off:off + w] = v
        bt[:, _BIAS_COLS["c"][0]] = coef
        m = {
            "xT": xT,
            "encT": np.ascontiguousarray(enc[b].T).astype(np_dt),
            "id128": np.eye(128, dtype=f32).astype(np_dt),
            "ones_col": np.ones((128, 1), f32).astype(np_dt),
            "ones_row": np.ones((1, 128), f32).astype(np_dt),
            "wqkv": wqkv, "wca": wca,
            "ow": inp["sa_o_w"].astype(np_dt), "o2w": inp["ca_o_w"].astype(np_dt),
            "biases": bt,
            "w13": w13.astype(np_dt),
            "w2": np.ascontiguousarray(inp["moe_w2"][e]).astype(np_dt),
        }
        if mode != "f32":
            m["xT_st"] = xT.astype(np_dt)
        if any(cc == 1 for cc in sa_cls):
            m["maskT"] = maskT0[b].astype(np_dt)
        if any(cc == 1 for cc in ca_cls):
            m["encmaskT"] = encmaskT0[b].astype(np_dt)
        in_maps.append(m)

    kw = {}
    if _TRACE_DIR:
        kw = dict(trace=True, tmpdir=_TRACE_DIR, trace_cores=[0])
    res = bass_utils.run_bass_kernel_spmd(nc, in_maps, core_ids=list(range(8)), **kw)
    global _LAST_EXEC_NS
    _LAST_EXEC_NS = res.exec_time_ns
    return np.stack([
        res.results[2 * b]["out_res"].T
        + res.results[2 * b]["out_ffn"]
        + res.results[2 * b + 1]["out_ffn"]
        for b in range(B)
    ]).astype(f32)



# revision 2
# speedup vs baseline: 1.0460x; 1.0460x over previous
"""MBart MoE decoder layer on 8 trn2 NeuronCores.

Sharding: 8 cores = 8 (sequence, expert-slot) pairs. Core c handles
sequence b=c//2, expert slot j=c%2 (each sequence is lang-routed to at
most 2 distinct experts; routing is computed on the host from `langs`).
Each core computes the full attention path for its sequence (replicated
across the pair) and one expert FFN over all 256 tokens; the host sums
the pair's partial outputs (expert-sharded combine) and transposes back
to token-major. Expert weights are gathered per-core on the host, so a
core only receives the one expert it needs.

On-device layout is feature-major [D, tokens]: projections take weights
as lhsT (feature-major out) or activations as lhsT (token-major out), so
no activation transposes are needed anywhere. LN gains/biases are folded
into the downstream weights on the host; softmax uses transposed scores
[keys, queries] with the attention mask added via an identity-matmul
into PSUM (host classifies each 128x128 mask block as zero / add / skip,
so causal dead blocks are never computed) and denominators accumulated
via a ones-matmul, then broadcast over partitions with a rank-1 matmul;
the per-token reciprocal runs on the vector engine (reciprocal_approx_
fast) so the scalar engine keeps its exp table loaded across the whole
attend (ACT table reloads cost ~1.3us each).

DMA strategy: weights/inputs load as one packed [128, nchunk, W] tile
per tensor (single descriptor set, sync queue), while the big MoE
weights stream on the otherwise-idle gpsimd queue: w2 (fp8, resident in
SBUF from t~0) and w13 double-buffered 2MB groups. fp8 is used only for
the moving-operand w2 down-projection (quantization error there is
diluted by the residual path).
"""

import os
import sys
from contextlib import ExitStack

for _p in ("/opt/trn_rl_repo",):
    if _p not in sys.path:
        sys.path.append(_p)

import numpy as np
import ml_dtypes

import concourse.bass as bass
import concourse.tile as tile
import concourse.mybir as mybir
from concourse import bacc, bass_utils

B, S, SK = 4, 256, 512
D, NH, NKV, HD = 1024, 16, 4, 64
DE, NE = 4096, 8
LN_EPS = 1e-5
REP = NH // NKV
DC = D // 128    # 8 feature chunks
FC = DE // 128   # 32 ffn chunks
SC = S // 128    # 2 self-attn key chunks
KC = SK // 128   # 4 cross-attn key chunks
QC = S // 128    # 2 query halves
KVW = NKV * HD   # 256
GRP = 4          # ffn chunks per MoE weight group
NGRP = FC // GRP

MODE = os.environ.get("KERNEL_MM_DTYPE", "bf16")  # "bf16" | "f32r" | "f32"
W2_FP8 = os.environ.get("KERNEL_W2_FP8", "1") == "1"

_CACHE: dict = {}
_TRACE_DIR = None   # set by test harness for profiling runs
_LAST_EXEC_NS = None

# packed attention-weight column layout: qw | kw(dup) | vw
W_Q, W_K, W_V = 0, D, D + 2 * KVW
WPACK = D + 2 * KVW + KVW  # 1792

# packed per-partition bias column layout
_BIAS_COLS = {}
_off = 0
for _n, _w in [("qb", DC), ("kb", 4), ("vb", KVW), ("ob", DC),
               ("q2b", DC), ("k2b", 4), ("v2b", KVW), ("o2b", DC),
               ("b1", FC), ("b3", FC), ("c", 1)]:
    _BIAS_COLS[_n] = (_off, _w)
    _off += _w
BIAS_W = _off


def _build(mode, sa_cls, ca_cls):
    """sa_cls/ca_cls: block classes per (kc, qhalf): 0=no-mask, 1=mask-add,
    2=fully-masked(skip)."""
    st = {"bf16": mybir.dt.bfloat16, "f32r": mybir.dt.float32r,
          "f32": mybir.dt.float32}[mode]
    f32 = mybir.dt.float32
    w2_dt = mybir.dt.float8e4 if (mode == "bf16" and W2_FP8) else st
    same_st = mode == "f32"
    A = mybir.ActivationFunctionType
    OP = mybir.AluOpType

    nc = bacc.Bacc("TRN2", target_bir_lowering=False, debug=False, num_devices=8)

    def mm(psum, lhsT, rhs, start, stop):
        nc.tensor.matmul(psum, lhsT, rhs, start=start, stop=stop)

    di = {}

    def din(name, shape, dtype=None):
        di[name] = nc.dram_tensor(name, list(shape), dtype or st, kind="ExternalInput")
        return di[name]

    din("xT", (D, S), f32)
    if not same_st:
        din("xT_st", (D, S))
    din("encT", (D, SK))
    need_samask = any(c == 1 for c in sa_cls)
    need_camask = any(c == 1 for c in ca_cls)
    if need_samask:
        din("maskT", (S, S))
    if need_camask:
        din("encmaskT", (SK, S))
    din("id128", (128, 128))
    din("wqkv", (D, WPACK))
    din("wca", (D, WPACK))
    din("ow", (D, D))
    din("o2w", (D, D))
    din("biases", (128, BIAS_W), f32)
    din("w13", (D, 2 * DE))   # interleaved per group: [w1 g-cols | w3 g-cols]
    din("w2", (DE, D), w2_dt)
    out_res = nc.dram_tensor("out_res", [D, S], f32, kind="ExternalOutput")
    out_ffn = nc.dram_tensor("out_ffn", [S, D], f32, kind="ExternalOutput")

    def packed(dram):
        """[nchunk*128, W] dram -> [128, nchunk, W] AP."""
        return dram.ap().rearrange("(k p) c -> p k c", p=128)

    with tile.TileContext(nc) as tc, ExitStack() as ctx:
        cp = ctx.enter_context(tc.tile_pool(name="consts", bufs=1))
        pers = ctx.enter_context(tc.tile_pool(name="pers", bufs=1))

        ones128 = cp.tile([128, 1], st, tag="ones128", name="ones128")
        nc.vector.memset(ones128, 1.0)
        ones1r = cp.tile([1, 128], st, tag="ones1r", name="ones1r")
        nc.vector.memset(ones1r, 1.0)
        eps_t = cp.tile([128, 1], f32, tag="eps_t", name="eps_t")
        nc.vector.memset(eps_t, LN_EPS)
        id128 = cp.tile([128, 128], st, tag="id128", name="id128")
        nc.sync.dma_start(id128[:], di["id128"].ap())
        maskT = encmaskT = None
        if need_samask:
            maskT = cp.tile([128, SC, S], st, tag="maskT", name="maskT")
            nc.sync.dma_start(maskT[:], packed(di["maskT"]))
        if need_camask:
            encmaskT = cp.tile([128, KC, S], st, tag="encmaskT", name="encmaskT")
            nc.sync.dma_start(encmaskT[:], packed(di["encmaskT"]))

        bias_t = cp.tile([128, BIAS_W], f32, tag="bias_t", name="bias_t")
        nc.sync.dma_start(bias_t[:], di["biases"].ap())

        # MoE down-proj weights: resident in SBUF, streamed on the idle
        # gpsimd queue from t~0 so the whole down-proj runs back-to-back.
        w2t = pers.tile([128, FC, D], w2_dt, tag="w2t", name="w2t")
        nc.gpsimd.dma_start(w2t[:], packed(di["w2"]))

        def bias(nm):
            off, w = _BIAS_COLS[nm]
            return bias_t[:, off:off + w]

        def load_packed(dram, nchunk, width, tag, pool, dtype=st):
            t = pool.tile([128, nchunk, width], dtype, tag=tag, name=tag)
            nc.sync.dma_start(t[:], packed(dram))
            return [t[:, k, :] for k in range(nchunk)]

        def layernorm(src_f32, src_st, out_tag, pool):
            """src: DC chunks [128,S] f32 (+st copies). Returns DC normalized
            chunks [128,S] st (gain/bias folded downstream by host)."""
            with tc.tile_pool(name=f"{out_tag}_lt", bufs=2) as lp, \
                 tc.tile_pool(name=f"{out_tag}_lp", bufs=1, space="PSUM") as sp, \
                 tc.tile_pool(name=f"{out_tag}_lb", bufs=1, space="PSUM") as bp:
                sum_ps = sp.tile([1, S], f32, tag="lnsum", name="lnsum")
                sq_ps = sp.tile([1, S], f32, tag="lnsq", name="lnsq")
                for k in range(DC):
                    sq = lp.tile([128, S], st, tag="lnsqt", name="lnsqt")
                    nc.vector.tensor_tensor(sq[:], src_f32[k][:], src_f32[k][:],
                                            OP.mult)
                    mm(sum_ps[:], ones128[:], src_st[k][:], k == 0, k == DC - 1)
                    mm(sq_ps[:], ones128[:], sq[:], k == 0, k == DC - 1)
                s_sb = lp.tile([1, S], st, tag="ln_ssb", name="ln_ssb")
                nc.vector.tensor_single_scalar(s_sb[:], sum_ps[:], 1.0 / D, OP.mult)
                q_sb = lp.tile([1, S], st, tag="ln_qsb", name="ln_qsb")
                nc.vector.tensor_single_scalar(q_sb[:], sq_ps[:], 1.0 / D, OP.mult)
                s_bc = bp.tile([128, S], f32, tag="ln_sbc", name="ln_sbc")
                q_bc = bp.tile([128, S], f32, tag="ln_qbc", name="ln_qbc")
                mm(s_bc[:], ones1r[:], s_sb[:], True, True)   # mean, bcast
                mm(q_bc[:], ones1r[:], q_sb[:], True, True)   # E[x^2], bcast
                # full-lane stats math on [128,S]
                s_sbuf = lp.tile([128, S], f32, tag="ln_ssbuf", name="ln_ssbuf")
                nc.vector.tensor_copy(s_sbuf[:], s_bc[:])
                var = lp.tile([128, S], f32, tag="ln_var", name="ln_var")
                nc.vector.scalar_tensor_tensor(var[:], s_bc[:], 0.0, s_sbuf[:],
                                               OP.bypass, OP.mult)
                nc.vector.tensor_sub(var[:], q_bc[:], var[:])
                v_t = lp.tile([128, S], f32, tag="ln_vt", name="ln_vt")
                nc.scalar.activation(v_t[:], var[:], A.Abs_reciprocal_sqrt,
                                     bias=eps_t[:])
                u_t = lp.tile([128, S], f32, tag="ln_ut", name="ln_ut")
                nc.vector.tensor_tensor(u_t[:], s_sbuf[:], v_t[:], OP.mult)
                outs = []
                for k in range(DC):
                    o = pool.tile([128, S], st, tag=f"{out_tag}{k}",
                                  name=f"{out_tag}{k}")
                    nc.vector.tensor_tensor(o[:], src_f32[k][:], v_t[:], OP.mult)
                    nc.vector.tensor_sub(o[:], o[:], u_t[:])
                    outs.append(o)
                return outs

        def cast_st(src, tag, pool):
            if same_st:
                return src
            outs = []
            for k, t in enumerate(src):
                o = pool.tile([128, S], st, tag=f"{tag}{k}",
                              name=f"{tag}{k}")
                nc.vector.tensor_copy(o[:], t[:])
                outs.append(o)
            return outs

        def project_fm(w_slices, rhs_chunks, nout, bias_ap, out_tag, pool,
                       extra=None, out_dt=None, width=None):
            """out^T[dout_chunk] = sum_k w_slices[k][:, m*128:...].T @ rhs[k]."""
            W = width or S
            outs = []
            with tc.tile_pool(name=f"{out_tag}_ps", bufs=3, space="PSUM") as pp:
                for mI in range(nout):
                    ps = pp.tile([128, W], f32, tag="proj", name="proj")
                    for k in range(DC):
                        mm(ps[:], w_slices[k][:, mI * 128:(mI + 1) * 128],
                           rhs_chunks[k][:], k == 0, k == DC - 1)
                    o = pool.tile([128, W], out_dt or st, tag=f"{out_tag}{mI}",
                                  name=f"{out_tag}{mI}")
                    if extra is not None:
                        extra(mI, ps, o)
                    elif bias_ap is not None:
                        nc.vector.tensor_scalar(o[:], ps[:],
                                                bias_ap[:, mI:mI + 1], None,
                                                OP.add)
                    else:
                        nc.vector.tensor_copy(o[:], ps[:])
                    outs.append(o)
            return outs

        def project_tm(act_chunks, w_slices, ntok, bias_bcast, out_tag, pool):
            """token-major V with a ones column appended per kv head:
            out[tok_chunk] = [V_kv | 1] blocks of 65 columns."""
            outs = []
            with tc.tile_pool(name=f"{out_tag}_ps", bufs=3, space="PSUM") as pp:
                for t in range(ntok):
                    ps = pp.tile([128, KVW], f32, tag="projtm", name="projtm")
                    for k in range(DC):
                        mm(ps[:], act_chunks[k][:, t * 128:(t + 1) * 128],
                           w_slices[k][:], k == 0, k == DC - 1)
                    o = pool.tile([128, NKV, HD + 1], st, tag=f"{out_tag}{t}",
                                  name=f"{out_tag}{t}")
                    nc.vector.tensor_add(
                        o[:, :, 0:HD],
                        ps[:].rearrange("p (kv d) -> p kv d", kv=NKV),
                        bias_bcast[:].rearrange("p (kv d) -> p kv d", kv=NKV))
                    for kv in range(NKV):
                        nc.vector.tensor_copy(o[:, kv, HD:HD + 1], ones128[:])
                    outs.append(o)
            return outs

        def attend(qT, kT, vtm, n_kc, mask_tile, cls, out_tag, pool):
            """Transposed-score attention. cls[kc*QC + qh] in {0,1,2}.
            vtm blocks are [V_kv | ones] of 65 cols, so each O' matmul also
            accumulates the softmax denominator into row 64."""
            outs = []
            # per kc: active query range (contiguous union of non-skip halves)
            qr = []
            for kc in range(n_kc):
                act = [qh for qh in range(QC) if cls[kc * QC + qh] != 2]
                assert act and act == list(range(act[0], act[-1] + 1))
                qr.append((act[0] * 128, (act[-1] + 1) * 128))
            with tc.tile_pool(name=f"{out_tag}_sp", bufs=3, space="PSUM") as stp, \
                 tc.tile_pool(name=f"{out_tag}_op", bufs=2, space="PSUM") as opp, \
                 tc.tile_pool(name=f"{out_tag}_bp", bufs=1, space="PSUM") as bpp, \
                 tc.tile_pool(name=f"{out_tag}_et", bufs=6) as epool, \
                 tc.tile_pool(name=f"{out_tag}_dt", bufs=3) as dpool:
                for c in range(DC):
                    o_ps_h = [opp.tile([65, S], f32, tag=f"oph{hh}",
                                       name=f"oph{hh}") for hh in range(2)]
                    kv = (2 * c) // REP      # same kv head for both of the pair
                    for kc in range(n_kc):
                        q0, q1 = qr[kc]
                        adds = [q for q in range(QC) if cls[kc * QC + q] == 1]
                        st_h = []
                        e_h = []
                        for hh in range(2):
                            qh_ap = qT[c][hh * 64:(hh + 1) * 64, :]
                            kh = kT[kv][hh * 64:(hh + 1) * 64, :]
                            st_ps = stp.tile([128, S], f32, tag="st",
                                             name="st")
                            mm(st_ps[:, q0:q1], kh[:, kc * 128:(kc + 1) * 128],
                               qh_ap[:, q0:q1], True, not adds)
                            st_h.append(st_ps)
                        for hh in range(2):
                            for i, q in enumerate(adds):
                                mm(st_h[hh][:, q * 128:(q + 1) * 128], id128[:],
                                   mask_tile[:, kc, q * 128:(q + 1) * 128],
                                   False, i == len(adds) - 1)
                        for hh in range(2):
                            e = epool.tile([128, S], st, tag="e", name="e")
                            nc.scalar.activation(e[:, q0:q1],
                                                 st_h[hh][:, q0:q1], A.Exp)
                            e_h.append(e)
                        for hh in range(2):
                            mm(o_ps_h[hh][:, q0:q1],
                               vtm[kc][:, kv, :], e_h[hh][:, q0:q1],
                               kc == 0, kc == n_kc - 1)
                    den_pair = dpool.tile([1, 2 * S], st, tag="den_pair",
                                          name="den_pair")
                    for hh in range(2):
                        nc.vector.tensor_copy(den_pair[:, hh * S:(hh + 1) * S],
                                              o_ps_h[hh][64:65, :])
                    r_ps = bpp.tile([128, 2 * S], f32, tag="rbc", name="rbc")
                    mm(r_ps[:], ones1r[:], den_pair[:], True, True)
                    # 1/x on the vector engine: keeps ACT's exp table loaded
                    # (an exp<->rsqrt table swap costs ~1.3us each way)
                    rbi = dpool.tile([128, 2 * S], f32, tag="rbi", name="rbi")
                    nc.vector.reciprocal_approx_fast(rbi[:], r_ps[:])
                    o = pool.tile([128, S], st, tag=f"{out_tag}{c}",
                                  name=f"{out_tag}{c}")
                    for hh in range(2):
                        nc.vector.tensor_tensor(
                            o[hh * 64:(hh + 1) * 64, :], o_ps_h[hh][0:64, :],
                            rbi[hh * 64:(hh + 1) * 64, hh * S:(hh + 1) * S],
                            OP.mult)
                    outs.append(o)
            return outs

        h1t = pers.tile([128, DC, S], f32, tag="h1T", name="h1T")
        h2t = pers.tile([128, DC, S], f32, tag="h2T", name="h2T")
        h1 = [h1t[:, k, :] for k in range(DC)]
        h2 = [h2t[:, k, :] for k in range(DC)]

        # ---------------- self attention ----------------
        with tc.tile_pool(name="sa_acts", bufs=1) as sa:
            xT = load_packed(di["xT"], DC, S, "xT", sa, f32)
            xT_st = xT if same_st else load_packed(di["xT_st"], DC, S, "xTs", sa)
            n1 = layernorm(xT, xT_st, "n1T", sa)
            with tc.tile_pool(name="wqkvp", bufs=1) as wp:
                wt = load_packed(di["wqkv"], DC, WPACK, "wqkv", wp)
                qT = project_fm([t[:, W_Q:W_Q + D] for t in wt], n1, DC,
                                bias("qb"), "qT", sa)
                kT = project_fm([t[:, W_K:W_K + 2 * KVW] for t in wt], n1, 4,
                                bias("kb"), "kT", sa)
                v_tm = project_tm(n1, [t[:, W_V:W_V + KVW] for t in wt], SC,
                                  bias("vb"), "v_tm", sa)
            with tc.tile_pool(name="wop", bufs=1) as wp:
                ow_t = load_packed(di["ow"], DC, D, "ow", wp)
                sa_out = attend(qT, kT, v_tm, SC, maskT, sa_cls, "saT", sa)

                def o_epil(mI, ps, o):
                    nc.vector.scalar_tensor_tensor(o[:], ps[:],
                                                   bias("ob")[:, mI:mI + 1],
                                                   xT[mI][:], OP.add, OP.add)
                project_fm(ow_t, sa_out, DC, None, "h1w", _FixedPool(h1),
                           extra=o_epil, out_dt=f32)

        # ---------------- cross attention ----------------
        with tc.tile_pool(name="ca_acts", bufs=1) as ca:
            encT = load_packed(di["encT"], DC, SK, "encT", ca)
            h1_st = cast_st(h1, "h1s", ca)
            with tc.tile_pool(name="wcap", bufs=1) as wp:
                wt = load_packed(di["wca"], DC, WPACK, "wca", wp)
                k2T = project_fm([t[:, W_K:W_K + 2 * KVW] for t in wt], encT, 4,
                                 bias("k2b"), "k2T", ca, width=SK)
                v2_tm = project_tm(encT, [t[:, W_V:W_V + KVW] for t in wt], KC,
                                   bias("v2b"), "v2_tm", ca)
                n2 = layernorm(h1, h1_st, "n2T", ca)
                q2T = project_fm([t[:, W_Q:W_Q + D] for t in wt], n2, DC,
                                 bias("q2b"), "q2T", ca)
            with tc.tile_pool(name="wo2p", bufs=1) as wp:
                o2w_t = load_packed(di["o2w"], DC, D, "o2w", wp)
                ca_out = attend(q2T, k2T, v2_tm, KC, encmaskT, ca_cls, "caT", ca)

                def o2_epil(mI, ps, o):
                    nc.vector.scalar_tensor_tensor(o[:], ps[:],
                                                   bias("o2b")[:, mI:mI + 1],
                                                   h1[mI][:], OP.add, OP.add)
                project_fm(o2w_t, ca_out, DC, None, "h2w", _FixedPool(h2),
                           extra=o2_epil, out_dt=f32)

        # residual output (host: out_b = res.T + ffn_j0 + ffn_j1)
        nc.sync.dma_start(packed(out_res), h2t[:])

        # ---------------- MoE expert ----------------
        with tc.tile_pool(name="moe_acts", bufs=1) as mo:
            h2_st = cast_st(h2, "h2s", mo)
            n3 = layernorm(h2, h2_st, "n3T", mo)

            mT = [mo.tile([128, S], st, tag=f"mT{m}", name=f"mT{m}")
                  for m in range(FC)]
            with tc.tile_pool(name="w13p", bufs=2) as wp, \
                 tc.tile_pool(name="gh_ps", bufs=3, space="PSUM") as gp, \
                 tc.tile_pool(name="gelu_t", bufs=3) as gt:
                gw = GRP * 128
                w13ap = packed(di["w13"])
                for g in range(NGRP):
                    wg = wp.tile([128, DC, 2 * gw], st, tag="w13g",
                                 name="w13g")
                    nc.gpsimd.dma_start(
                        wg[:], w13ap[:, :, g * 2 * gw:(g + 1) * 2 * gw])
                    for mi in range(GRP):
                        mI = g * GRP + mi
                        g_ps = gp.tile([128, S], f32, tag="g_ps", name="g_ps")
                        h_ps = gp.tile([128, S], f32, tag="h_ps", name="h_ps")
                        for k in range(DC):
                            mm(g_ps[:], wg[:, k, mi * 128:(mi + 1) * 128],
                               n3[k][:], k == 0, k == DC - 1)
                        for k in range(DC):
                            mm(h_ps[:], wg[:, k, gw + mi * 128:gw + (mi + 1) * 128],
                               n3[k][:], k == 0, k == DC - 1)
                        ge = gt.tile([128, S], f32, tag="ge", name="ge")
                        nc.scalar.activation(ge[:], g_ps[:], A.Gelu,
                                             bias=bias("b1")[:, mI:mI + 1])
                        nc.vector.scalar_tensor_tensor(mT[mI][:], h_ps[:],
                                                       bias("b3")[:, mI:mI + 1],
                                                       ge[:], OP.add, OP.mult)

            # down-proj, token-major out: y[t,n] = sum_f M^T[f,t].T @ w2[f,n]
            with tc.tile_pool(name="y_ps", bufs=1, space="PSUM") as yp, \
                 tc.tile_pool(name="outp", bufs=3) as op_:
                y_ps = [[yp.tile([128, 512], f32, tag=f"y{t}{n}", name=f"y{t}{n}")
                         for n in range(2)] for t in range(QC)]
                for k2 in range(FC):
                    for t in range(QC):
                        for n in range(2):
                            mm(y_ps[t][n][:], mT[k2][:, t * 128:(t + 1) * 128],
                               w2t[:, k2, n * 512:(n + 1) * 512],
                               k2 == 0, k2 == FC - 1)
                for t in range(QC):
                    for n in range(2):
                        o = op_.tile([128, 512], f32, tag="o_out", name="o_out")
                        nc.vector.tensor_scalar_mul(o[:], y_ps[t][n][:],
                                                    bias("c")[:, 0:1])
                        nc.sync.dma_start(
                            out_ffn.ap()[t * 128:(t + 1) * 128,
                                         n * 512:(n + 1) * 512], o[:])

    nc.compile()
    return nc


class _FixedPool:
    """Adapter letting project_fm write into pre-allocated tile slices."""

    def __init__(self, tiles):
        self._tiles = list(tiles)
        self._i = 0

    def tile(self, shape, dtype, tag=None, name=None):
        t = self._tiles[self._i]
        self._i += 1
        return t


def _routing(langs):
    """Per-sequence expert slots [(expert_idx, coef) x2], matching the
    reference: coef[e,b] = any(langs[b]==4+e) * (1/count(langs[b]>3))."""
    langs = np.asarray(langs)
    slots = []
    for b in range(langs.shape[0]):
        row = [int(v) for v in langs[b]]
        cnt = sum(1 for v in row if v > 3)
        rw = 1.0 if cnt == 0 else 1.0 / cnt
        seen = []
        for v in row:
            if v > 3 and 0 <= v - 4 < NE and (v - 4) not in seen:
                seen.append(v - 4)
        sl = [(e, rw) for e in seen]
        while len(sl) < 2:
            sl.append((0, 0.0))
        slots.append(sl[:2])
    return slots


def _mask_classes(maskT, n_kc):
    """Classify each [128 keys x 128 queries] block of a transposed mask:
    0 all-zero (no add), 1 general (add), 2 fully masked (skip compute).
    Keeps at least one active key block per query and contiguous active
    ranges per key chunk."""
    cls = []
    for kc in range(n_kc):
        for qh in range(QC):
            blk = maskT[kc * 128:(kc + 1) * 128, qh * 128:(qh + 1) * 128]
            if np.all(blk == 0):
                cls.append(0)
            elif np.all(blk <= -1e8):
                cls.append(2)
            else:
                cls.append(1)
    for qh in range(QC):
        if all(cls[kc * QC + qh] == 2 for kc in range(n_kc)):
            for kc in range(n_kc):
                cls[kc * QC + qh] = 1
    for kc in range(n_kc):
        act = [q for q in range(QC) if cls[kc * QC + q] != 2]
        if not act or act != list(range(act[0], act[-1] + 1)):
            for q in range(QC):
                if cls[kc * QC + q] == 2:
                    cls[kc * QC + q] = 1
    return tuple(cls)


def kernel(**inputs):
    mode = MODE
    np_dt = ml_dtypes.bfloat16 if mode == "bf16" else np.float32
    f32 = np.float32

    inp = {k: np.asarray(v) for k, v in inputs.items()}
    x = inp["hidden_states"].astype(f32)
    enc = inp["encoder_hidden_states"].astype(f32)
    mask = inp["attention_mask"].astype(f32)
    encmask = inp["encoder_attention_mask"].astype(f32)
    g1, b1 = inp["ln1_g"].astype(f32), inp["ln1_b"].astype(f32)
    g2, b2 = inp["ln2_g"].astype(f32), inp["ln2_b"].astype(f32)
    g3, b3 = inp["ln3_g"].astype(f32), inp["ln3_b"].astype(f32)

    def dup_kv(w):
        return np.concatenate([np.tile(w[:, 64 * j:64 * (j + 1)], (1, 2))
                               for j in range(NKV)], axis=1)

    def dup_kv_b(v):
        return np.concatenate([np.tile(v[64 * j:64 * (j + 1)], 2)
                               for j in range(NKV)])

    sc = HD ** -0.5
    qw_f = g1[:, None] * inp["sa_q_w"] * sc
    qb_f = (b1 @ inp["sa_q_w"] + inp["sa_q_b"]) * sc
    kw_f = dup_kv(g1[:, None] * inp["sa_k_w"])
    kb_f = dup_kv_b(b1 @ inp["sa_k_w"] + inp["sa_k_b"])
    vw_f = g1[:, None] * inp["sa_v_w"]
    vb_f = b1 @ inp["sa_v_w"] + inp["sa_v_b"]
    q2w_f = g2[:, None] * inp["ca_q_w"] * sc
    q2b_f = (b2 @ inp["ca_q_w"] + inp["ca_q_b"]) * sc
    k2w_f = dup_kv(inp["ca_k_w"])
    k2b_f = dup_kv_b(inp["ca_k_b"])
    w1_f = inp["moe_w1"] * g3[None, :, None]
    b1_f = np.einsum("d,edf->ef", b3, inp["moe_w1"]).astype(f32)
    w3_f = inp["moe_w3"] * g3[None, :, None]
    b3_f = np.einsum("d,edf->ef", b3, inp["moe_w3"]).astype(f32)

    maskT0 = np.ascontiguousarray(mask[:, 0].transpose(0, 2, 1))     # [B,S,S]
    encmaskT0 = np.ascontiguousarray(encmask[:, 0].transpose(0, 2, 1))
    sa_cls = _mask_classes(maskT0[0], SC)
    ca_cls = _mask_classes(encmaskT0[0], KC)
    for b in range(1, B):
        if _mask_classes(maskT0[b], SC) != sa_cls or \
           _mask_classes(encmaskT0[b], KC) != ca_cls:
            sa_cls = tuple(1 for _ in range(SC * QC))
            ca_cls = tuple(1 for _ in range(KC * QC))
            break

    key = (mode, sa_cls, ca_cls)
    if key not in _CACHE:
        _CACHE[key] = _build(mode, sa_cls, ca_cls)
    nc = _CACHE[key]

    def col128(v):
        return np.asarray(v, f32).reshape(-1, 128).T

    slots = _routing(inp["langs"])
    wqkv = np.concatenate([qw_f, kw_f, vw_f], axis=1).astype(np_dt)
    wca = np.concatenate([q2w_f, k2w_f, inp["ca_v_w"]], axis=1).astype(np_dt)

    bias_common = np.zeros((128, BIAS_W), f32)
    for nm, v in [("qb", col128(qb_f)), ("kb", col128(kb_f)),
                  ("vb", np.broadcast_to(vb_f.astype(f32), (128, KVW))),
                  ("ob", col128(inp["sa_o_b"])),
                  ("q2b", col128(q2b_f)), ("k2b", col128(k2b_f)),
                  ("v2b", np.broadcast_to(inp["ca_v_b"].astype(f32), (128, KVW))),
                  ("o2b", col128(inp["ca_o_b"]))]:
        off, w = _BIAS_COLS[nm]
        bias_common[:, off:off + w] = v

    w2_np_dt = ml_dtypes.float8_e4m3 if (mode == "bf16" and W2_FP8) else np_dt

    in_maps = []
    for c in range(8):
        b, j = c // 2, c % 2
        e, coef = slots[b][j]
        xT = np.ascontiguousarray(x[b].T)
        # interleave w1/w3 by group: [w1 grp g | w3 grp g] blocks of 512 cols
        gw = GRP * 128
        w13 = np.empty((D, 2 * DE), f32)
        for g in range(NGRP):
            w13[:, g * 2 * gw:g * 2 * gw + gw] = w1_f[e][:, g * gw:(g + 1) * gw]
            w13[:, g * 2 * gw + gw:(g + 1) * 2 * gw] = w3_f[e][:, g * gw:(g + 1) * gw]
        bt = bias_common.copy()
        for nm, v in [("b1", col128(b1_f[e])), ("b3", col128(b3_f[e]))]:
            off, w = _BIAS_COLS[nm]
            bt[:, off:off + w] = v
        bt[:, _BIAS_COLS["c"][0]] = coef
        m = {
            "xT": xT,
            "encT": np.ascontiguousarray(enc[b].T).astype(np_dt),
            "id128": np.eye(128, dtype=f32).astype(np_dt),
            "wqkv": wqkv, "wca": wca,
            "ow": inp["sa_o_w"].astype(np_dt), "o2w": inp["ca_o_w"].astype(np_dt),
            "biases": bt,
            "w13": w13.astype(np_dt),
            "w2": np.ascontiguousarray(inp["moe_w2"][e]).astype(w2_np_dt),
        }
        if mode != "f32":
            m["xT_st"] = xT.astype(np_dt)
        if any(cc == 1 for cc in sa_cls):
            m["maskT"] = maskT0[b].astype(np_dt)
        if any(cc == 1 for cc in ca_cls):
            m["encmaskT"] = encmaskT0[b].astype(np_dt)
        in_maps.append(m)

    kw = {}
    if _TRACE_DIR:
        kw = dict(trace=True, tmpdir=_TRACE_DIR, trace_cores=[0])
    res = bass_utils.run_bass_kernel_spmd(nc, in_maps, core_ids=list(range(8)), **kw)
    global _LAST_EXEC_NS
    _LAST_EXEC_NS = res.exec_time_ns
    return np.stack([
        res.results[2 * b]["out_res"].T
        + res.results[2 * b]["out_ffn"]
        + res.results[2 * b + 1]["out_ffn"]
        for b in range(B)
    ]).astype(f32)


# revision 13
# speedup vs baseline: 1.2711x; 1.2151x over previous
"""MBart MoE decoder layer on 8 trn2 NeuronCores.

Sharding: 8 cores = 8 (sequence, expert-slot) pairs. Core c handles
sequence b=c//2, expert slot j=c%2 (each sequence is lang-routed to at
most 2 distinct experts; routing is computed on the host from `langs`).
Each core computes the full attention path for its sequence (replicated
across the pair) and one expert FFN over all 256 tokens; the host sums
the pair's partial outputs (expert-sharded combine) and transposes back
to token-major. Expert weights are gathered per-core on the host, so a
core only receives the one expert it needs.

On-device layout is feature-major [D, tokens]: projections take weights
as lhsT (feature-major out) or activations as lhsT (token-major out), so
no activation transposes are needed anywhere. LN gains/biases are folded
into the downstream weights on the host; softmax uses transposed scores
[keys, queries] with the attention mask added via an identity-matmul
into PSUM (host classifies each 128x128 mask block as zero / add / skip,
so causal dead blocks are never computed) and denominators accumulated
via a ones-matmul, then broadcast over partitions with a rank-1 matmul;
the per-token reciprocal runs on the vector engine (reciprocal_approx_
fast) so the scalar engine keeps its exp table loaded across the whole
attend (ACT table reloads cost ~1.3us each).

DMA strategy: weights/inputs load as one packed [128, nchunk, W] tile
per tensor (single descriptor set, sync queue), while the big MoE
weights stream on the otherwise-idle gpsimd queue: w2 (fp8, resident in
SBUF from t~0) and w13 double-buffered 2MB groups. fp8 is used only for
the moving-operand w2 down-projection (quantization error there is
diluted by the residual path).
"""

import os
import sys
from contextlib import ExitStack

for _p in ("/opt/trn_rl_repo",):
    if _p not in sys.path:
        sys.path.append(_p)

import numpy as np
import ml_dtypes

import concourse.bass as bass
import concourse.tile as tile
import concourse.mybir as mybir
from concourse import bacc, bass_utils

B, S, SK = 4, 256, 512
D, NH, NKV, HD = 1024, 16, 4, 64
DE, NE = 4096, 8
LN_EPS = 1e-5
REP = NH // NKV
DC = D // 128    # 8 feature chunks
FC = DE // 128   # 32 ffn chunks
SC = S // 128    # 2 self-attn key chunks
KC = SK // 128   # 4 cross-attn key chunks
QC = S // 128    # 2 query halves
KVW = NKV * HD   # 256
GRP = 4          # ffn chunks per MoE weight group
NGRP = FC // GRP

MODE = os.environ.get("KERNEL_MM_DTYPE", "bf16")  # "bf16" | "f32r" | "f32"
MOE_FP8 = os.environ.get("KERNEL_MOE_FP8", "1") == "1"
FP8_SCALE = 128.0  # e3m4 normals start at 0.25; 0.02-scale weights need the boost

_CACHE: dict = {}
_TRACE_DIR = None   # set by test harness for profiling runs
_LAST_EXEC_NS = None

# packed attention-weight column layout: qw | kw(dup) | vw
W_Q, W_K, W_V = 0, D, D + 2 * KVW
WPACK = D + 2 * KVW + KVW  # 1792

# packed per-partition bias column layout
_BIAS_COLS = {}
_off = 0
for _n, _w in [("qb", DC), ("kb", 4), ("vb", KVW), ("ob", DC),
               ("q2b", DC), ("k2b", 4), ("v2b", KVW), ("o2b", DC),
               ("b1", FC), ("b3", FC), ("c", 1)]:
    _BIAS_COLS[_n] = (_off, _w)
    _off += _w
BIAS_W = _off


def _build(mode, sa_cls, ca_cls):
    """sa_cls/ca_cls: block classes per (kc, qhalf): 0=no-mask, 1=mask-add,
    2=fully-masked(skip)."""
    st = {"bf16": mybir.dt.bfloat16, "f32r": mybir.dt.float32r,
          "f32": mybir.dt.float32}[mode]
    f32 = mybir.dt.float32
    moe_fp8 = mode == "bf16" and MOE_FP8
    moe_dt = mybir.dt.float8e3 if moe_fp8 else st
    same_st = mode == "f32"
    A = mybir.ActivationFunctionType
    OP = mybir.AluOpType

    nc = bacc.Bacc("TRN2", target_bir_lowering=False, debug=False, num_devices=8)

    def mm(psum, lhsT, rhs, start, stop):
        nc.tensor.matmul(psum, lhsT, rhs, start=start, stop=stop)

    di = {}

    def din(name, shape, dtype=None):
        di[name] = nc.dram_tensor(name, list(shape), dtype or st, kind="ExternalInput")
        return di[name]

    din("xT", (D, S), f32)
    if not same_st:
        din("xT_st", (D, S))
    din("encT", (D, SK))
    need_samask = any(c == 1 for c in sa_cls)
    need_camask = any(c == 1 for c in ca_cls)
    if need_samask:
        din("maskT", (S, S))
    if need_camask:
        din("encmaskT", (SK, S))
    din("id128", (128, 128))
    din("wqkv", (D, WPACK))
    din("wca", (D, WPACK))
    din("ow", (D, D))
    din("o2w", (D, D))
    din("biases", (128, BIAS_W), f32)
    din("w13", (D, 2 * DE), moe_dt)  # interleaved: [w1 grp g | w3 grp g]
    din("w2", (DE, D), moe_dt)
    out_res = nc.dram_tensor("out_res", [D, S], f32, kind="ExternalOutput")
    out_ffn = nc.dram_tensor("out_ffn", [S, D], f32, kind="ExternalOutput")

    def packed(dram):
        """[nchunk*128, W] dram -> [128, nchunk, W] AP."""
        return dram.ap().rearrange("(k p) c -> p k c", p=128)

    with tile.TileContext(nc) as tc, ExitStack() as ctx:
        cp = ctx.enter_context(tc.tile_pool(name="consts", bufs=1))
        pers = ctx.enter_context(tc.tile_pool(name="pers", bufs=1))

        ones128 = cp.tile([128, 1], st, tag="ones128", name="ones128")
        nc.vector.memset(ones128, 1.0)
        ones1r = cp.tile([1, 128], st, tag="ones1r", name="ones1r")
        nc.vector.memset(ones1r, 1.0)
        eps_t = cp.tile([128, 1], f32, tag="eps_t", name="eps_t")
        nc.vector.memset(eps_t, LN_EPS)
        id128 = cp.tile([128, 128], st, tag="id128", name="id128")
        nc.sync.dma_start(id128[:], di["id128"].ap())
        maskT = encmaskT = None
        if need_samask:
            maskT = cp.tile([128, SC, S], st, tag="maskT", name="maskT")
            nc.sync.dma_start(maskT[:], packed(di["maskT"]))
        if need_camask:
            encmaskT = cp.tile([128, KC, S], st, tag="encmaskT", name="encmaskT")
            nc.sync.dma_start(encmaskT[:], packed(di["encmaskT"]))

        bias_t = cp.tile([128, BIAS_W], f32, tag="bias_t", name="bias_t")
        nc.sync.dma_start(bias_t[:], di["biases"].ap())

        # MoE down-proj weights are SBUF-resident (fp8); the DMA issues from
        # the scalar queue after the SA exps so the transfer lands in the
        # DMA-idle SA-attend window instead of starving the critical-path
        # attention loads. w1/w3 stream as double-buffered fp8 groups.
        w2t = pers.tile([128, FC, D], moe_dt, tag="w2t", name="w2t")
        w13pool = ctx.enter_context(tc.tile_pool(name="w13p", bufs=2))
        w13ap = packed(di["w13"])
        gw = GRP * 128

        def w13_group(g, engine):
            wg = w13pool.tile([128, DC, 2 * gw], moe_dt, tag="w13g",
                              name="w13g")
            engine.dma_start(wg[:], w13ap[:, :, g * 2 * gw:(g + 1) * 2 * gw])
            return wg

        def bias(nm):
            off, w = _BIAS_COLS[nm]
            return bias_t[:, off:off + w]

        def load_packed(dram, nchunk, width, tag, pool, dtype=st, nsplit=1):
            t = pool.tile([128, nchunk, width], dtype, tag=tag, name=tag)
            ap = packed(dram)
            step = nchunk // nsplit
            for s in range(nsplit):
                nc.sync.dma_start(t[:, s * step:(s + 1) * step, :],
                                  ap[:, s * step:(s + 1) * step, :])
            return [t[:, k, :] for k in range(nchunk)]

        def layernorm(src_f32, src_st, out_tag, pool):
            """src: DC chunks [128,S] f32 (+st copies). Returns DC normalized
            chunks [128,S] st (gain/bias folded downstream by host)."""
            with tc.tile_pool(name=f"{out_tag}_lt", bufs=2) as lp, \
                 tc.tile_pool(name=f"{out_tag}_lp", bufs=1, space="PSUM") as sp, \
                 tc.tile_pool(name=f"{out_tag}_lb", bufs=1, space="PSUM") as bp:
                sum_ps = sp.tile([1, S], f32, tag="lnsum", name="lnsum")
                sq_ps = sp.tile([1, S], f32, tag="lnsq", name="lnsq")
                for k in range(DC):
                    sq = lp.tile([128, S], st, tag="lnsqt", name="lnsqt")
                    nc.vector.tensor_tensor(sq[:], src_f32[k][:], src_f32[k][:],
                                            OP.mult)
                    mm(sum_ps[:], ones128[:], src_st[k][:], k == 0, k == DC - 1)
                    mm(sq_ps[:], ones128[:], sq[:], k == 0, k == DC - 1)
                s_sb = lp.tile([1, S], st, tag="ln_ssb", name="ln_ssb")
                nc.vector.tensor_single_scalar(s_sb[:], sum_ps[:], 1.0 / D, OP.mult)
                q_sb = lp.tile([1, S], st, tag="ln_qsb", name="ln_qsb")
                nc.vector.tensor_single_scalar(q_sb[:], sq_ps[:], 1.0 / D, OP.mult)
                s_bc = bp.tile([128, S], f32, tag="ln_sbc", name="ln_sbc")
                q_bc = bp.tile([128, S], f32, tag="ln_qbc", name="ln_qbc")
                mm(s_bc[:], ones1r[:], s_sb[:], True, True)   # mean, bcast
                mm(q_bc[:], ones1r[:], q_sb[:], True, True)   # E[x^2], bcast
                # full-lane stats math on [128,S]
                s_sbuf = lp.tile([128, S], f32, tag="ln_ssbuf", name="ln_ssbuf")
                nc.vector.tensor_copy(s_sbuf[:], s_bc[:])
                var = lp.tile([128, S], f32, tag="ln_var", name="ln_var")
                nc.vector.scalar_tensor_tensor(var[:], s_bc[:], 0.0, s_sbuf[:],
                                               OP.bypass, OP.mult)
                nc.vector.tensor_sub(var[:], q_bc[:], var[:])
                v_t = lp.tile([128, S], f32, tag="ln_vt", name="ln_vt")
                nc.scalar.activation(v_t[:], var[:], A.Abs_reciprocal_sqrt,
                                     bias=eps_t[:])
                u_t = lp.tile([128, S], f32, tag="ln_ut", name="ln_ut")
                nc.vector.tensor_tensor(u_t[:], s_sbuf[:], v_t[:], OP.mult)
                outs = []
                for k in range(DC):
                    o = pool.tile([128, S], st, tag=f"{out_tag}{k}",
                                  name=f"{out_tag}{k}")
                    nc.vector.tensor_tensor(o[:], src_f32[k][:], v_t[:], OP.mult)
                    nc.vector.tensor_sub(o[:], o[:], u_t[:])
                    outs.append(o)
                return outs

        def cast_st(src, tag, pool):
            if same_st:
                return src
            outs = []
            for k, t in enumerate(src):
                o = pool.tile([128, S], st, tag=f"{tag}{k}",
                              name=f"{tag}{k}")
                nc.vector.tensor_copy(o[:], t[:])
                outs.append(o)
            return outs

        def project_fm(w_slices, rhs_chunks, nout, bias_ap, out_tag, pool,
                       extra=None, out_dt=None, width=None):
            """out^T[dout_chunk] = sum_k w_slices[k][:, m*128:...].T @ rhs[k]."""
            W = width or S
            outs = []
            with tc.tile_pool(name=f"{out_tag}_ps", bufs=3, space="PSUM") as pp:
                for mI in range(nout):
                    ps = pp.tile([128, W], f32, tag="proj", name="proj")
                    for k in range(DC):
                        mm(ps[:], w_slices[k][:, mI * 128:(mI + 1) * 128],
                           rhs_chunks[k][:], k == 0, k == DC - 1)
                    o = pool.tile([128, W], out_dt or st, tag=f"{out_tag}{mI}",
                                  name=f"{out_tag}{mI}")
                    if extra is not None:
                        extra(mI, ps, o)
                    elif bias_ap is not None:
                        nc.vector.tensor_scalar(o[:], ps[:],
                                                bias_ap[:, mI:mI + 1], None,
                                                OP.add)
                    else:
                        nc.vector.tensor_copy(o[:], ps[:])
                    outs.append(o)
            return outs

        def project_tm(act_chunks, w_slices, ntok, bias_bcast, out_tag, pool):
            """token-major V with a ones column appended per kv head:
            out[tok_chunk] = [V_kv | 1] blocks of 65 columns."""
            outs = []
            with tc.tile_pool(name=f"{out_tag}_ps", bufs=3, space="PSUM") as pp:
                for t in range(ntok):
                    ps = pp.tile([128, KVW], f32, tag="projtm", name="projtm")
                    for k in range(DC):
                        mm(ps[:], act_chunks[k][:, t * 128:(t + 1) * 128],
                           w_slices[k][:], k == 0, k == DC - 1)
                    o = pool.tile([128, NKV, HD + 1], st, tag=f"{out_tag}{t}",
                                  name=f"{out_tag}{t}")
                    nc.vector.tensor_add(
                        o[:, :, 0:HD],
                        ps[:].rearrange("p (kv d) -> p kv d", kv=NKV),
                        bias_bcast[:].rearrange("p (kv d) -> p kv d", kv=NKV))
                    for kv in range(NKV):
                        nc.vector.tensor_copy(o[:, kv, HD:HD + 1], ones128[:])
                    outs.append(o)
            return outs

        def attend(qT, kT, vtm, n_kc, mask_tile, cls, out_tag, pool):
            """Transposed-score attention. cls[kc*QC + qh] in {0,1,2}.
            vtm blocks are [V_kv | ones] of 65 cols, so each O' matmul also
            accumulates the softmax denominator into row 64."""
            outs = []
            # per kc: active query range (contiguous union of non-skip halves)
            qr = []
            for kc in range(n_kc):
                act = [qh for qh in range(QC) if cls[kc * QC + qh] != 2]
                assert act and act == list(range(act[0], act[-1] + 1))
                qr.append((act[0] * 128, (act[-1] + 1) * 128))
            with tc.tile_pool(name=f"{out_tag}_sp", bufs=3, space="PSUM") as stp, \
                 tc.tile_pool(name=f"{out_tag}_op", bufs=2, space="PSUM") as opp, \
                 tc.tile_pool(name=f"{out_tag}_bp", bufs=1, space="PSUM") as bpp, \
                 tc.tile_pool(name=f"{out_tag}_et", bufs=6) as epool, \
                 tc.tile_pool(name=f"{out_tag}_dt", bufs=3) as dpool:
                for c in range(DC):
                    o_ps_h = [opp.tile([65, S], f32, tag=f"oph{hh}",
                                       name=f"oph{hh}") for hh in range(2)]
                    kv = (2 * c) // REP      # same kv head for both of the pair
                    for kc in range(n_kc):
                        q0, q1 = qr[kc]
                        adds = [q for q in range(QC) if cls[kc * QC + q] == 1]
                        st_h = []
                        e_h = []
                        for hh in range(2):
                            qh_ap = qT[c][hh * 64:(hh + 1) * 64, :]
                            kh = kT[kv][hh * 64:(hh + 1) * 64, :]
                            st_ps = stp.tile([128, S], f32, tag="st",
                                             name="st")
                            mm(st_ps[:, q0:q1], kh[:, kc * 128:(kc + 1) * 128],
                               qh_ap[:, q0:q1], True, not adds)
                            st_h.append(st_ps)
                        for hh in range(2):
                            for i, q in enumerate(adds):
                                mm(st_h[hh][:, q * 128:(q + 1) * 128], id128[:],
                                   mask_tile[:, kc, q * 128:(q + 1) * 128],
                                   False, i == len(adds) - 1)
                        for hh in range(2):
                            e = epool.tile([128, S], st, tag="e", name="e")
                            nc.scalar.activation(e[:, q0:q1],
                                                 st_h[hh][:, q0:q1], A.Exp)
                            e_h.append(e)
                        for hh in range(2):
                            mm(o_ps_h[hh][:, q0:q1],
                               vtm[kc][:, kv, :], e_h[hh][:, q0:q1],
                               kc == 0, kc == n_kc - 1)
                    den_pair = dpool.tile([1, 2 * S], st, tag="den_pair",
                                          name="den_pair")
                    for hh in range(2):
                        nc.vector.tensor_copy(den_pair[:, hh * S:(hh + 1) * S],
                                              o_ps_h[hh][64:65, :])
                    r_ps = bpp.tile([128, 2 * S], f32, tag="rbc", name="rbc")
                    mm(r_ps[:], ones1r[:], den_pair[:], True, True)
                    # 1/x on the vector engine: keeps ACT's exp table loaded
                    # (an exp<->rsqrt table swap costs ~1.3us each way)
                    rbi = dpool.tile([128, 2 * S], f32, tag="rbi", name="rbi")
                    nc.vector.reciprocal_approx_fast(rbi[:], r_ps[:])
                    o = pool.tile([128, S], st, tag=f"{out_tag}{c}",
                                  name=f"{out_tag}{c}")
                    for hh in range(2):
                        nc.vector.tensor_tensor(
                            o[hh * 64:(hh + 1) * 64, :], o_ps_h[hh][0:64, :],
                            rbi[hh * 64:(hh + 1) * 64, hh * S:(hh + 1) * S],
                            OP.mult)
                    outs.append(o)
            return outs

        h1t = pers.tile([128, DC, S], f32, tag="h1T", name="h1T")
        h2t = pers.tile([128, DC, S], f32, tag="h2T", name="h2T")
        h1 = [h1t[:, k, :] for k in range(DC)]
        h2 = [h2t[:, k, :] for k in range(DC)]

        cain = ctx.enter_context(tc.tile_pool(name="ca_in", bufs=1))

        # ---------------- self attention ----------------
        with tc.tile_pool(name="sa_acts", bufs=1) as sa:
            xT = load_packed(di["xT"], DC, S, "xT", sa, f32, nsplit=2)
            xT_st = xT if same_st else load_packed(di["xT_st"], DC, S, "xTs", sa)
            with tc.tile_pool(name="wqkvp", bufs=1) as wp:
                wt = load_packed(di["wqkv"], DC, WPACK, "wqkv", wp, nsplit=2)
                ow_t = load_packed(di["ow"], DC, D, "ow", wp)
                # cross-attn inputs prefetch behind the SA-critical loads
                encT = load_packed(di["encT"], DC, SK, "encT", cain)
                wt2 = load_packed(di["wca"], DC, WPACK, "wca", cain, nsplit=2)
                n1 = layernorm(xT, xT_st, "n1T", sa)
                qT = project_fm([t[:, W_Q:W_Q + D] for t in wt], n1, DC,
                                bias("qb"), "qT", sa)
                kT = project_fm([t[:, W_K:W_K + 2 * KVW] for t in wt], n1, 4,
                                bias("kb"), "kT", sa)
                v_tm = project_tm(n1, [t[:, W_V:W_V + KVW] for t in wt], SC,
                                  bias("vb"), "v_tm", sa)
                sa_out = attend(qT, kT, v_tm, SC, maskT, sa_cls, "saT", sa)
                # resident/prefetched MoE weights: issue on the scalar queue
                # here so the transfers land in the SA-attend DMA lull
                w13g01 = [w13_group(0, nc.scalar), w13_group(1, nc.scalar)]
                nc.scalar.dma_start(w2t[:], packed(di["w2"]))

                def o_epil(mI, ps, o):
                    nc.vector.scalar_tensor_tensor(o[:], ps[:],
                                                   bias("ob")[:, mI:mI + 1],
                                                   xT[mI][:], OP.add, OP.add)
                project_fm(ow_t, sa_out, DC, None, "h1w", _FixedPool(h1),
                           extra=o_epil, out_dt=f32)

        # ---------------- cross attention ----------------
        with tc.tile_pool(name="ca_acts", bufs=1) as ca:
            h1_st = cast_st(h1, "h1s", ca)
            with tc.tile_pool(name="wcap", bufs=1) as wp:
                wt = wt2
                k2T = project_fm([t[:, W_K:W_K + 2 * KVW] for t in wt], encT, 4,
                                 bias("k2b"), "k2T", ca, width=SK)
                v2_tm = project_tm(encT, [t[:, W_V:W_V + KVW] for t in wt], KC,
                                   bias("v2b"), "v2_tm", ca)
                n2 = layernorm(h1, h1_st, "n2T", ca)
                q2T = project_fm([t[:, W_Q:W_Q + D] for t in wt], n2, DC,
                                 bias("q2b"), "q2T", ca)
            with tc.tile_pool(name="wo2p", bufs=1) as wp:
                o2w_t = load_packed(di["o2w"], DC, D, "o2w", wp)
                ca_out = attend(q2T, k2T, v2_tm, KC, encmaskT, ca_cls, "caT", ca)

                def o2_epil(mI, ps, o):
                    nc.vector.scalar_tensor_tensor(o[:], ps[:],
                                                   bias("o2b")[:, mI:mI + 1],
                                                   h1[mI][:], OP.add, OP.add)
                project_fm(o2w_t, ca_out, DC, None, "h2w", _FixedPool(h2),
                           extra=o2_epil, out_dt=f32)

        # residual output (host: out_b = res.T + ffn_j0 + ffn_j1)
        nc.sync.dma_start(packed(out_res), h2t[:])

        # ---------------- MoE expert ----------------
        with tc.tile_pool(name="moe_acts", bufs=1) as mo:
            h2_st = cast_st(h2, "h2s", mo)
            n3 = layernorm(h2, h2_st, "n3T", mo)

            mT = [mo.tile([128, S], st, tag=f"mT{m}", name=f"mT{m}")
                  for m in range(FC)]
            ge_scale = 1.0 / FP8_SCALE if moe_fp8 else 1.0
            with tc.tile_pool(name="gh_ps", bufs=3, space="PSUM") as gp, \
                 tc.tile_pool(name="gelu_t", bufs=3) as gt:
                for g in range(NGRP):
                    wg = w13g01[g] if g < 2 else w13_group(g, nc.gpsimd)
                    for mi in range(GRP):
                        mI = g * GRP + mi
                        g_ps = gp.tile([128, S], f32, tag="g_ps", name="g_ps")
                        h_ps = gp.tile([128, S], f32, tag="h_ps", name="h_ps")
                        for k in range(DC):
                            mm(g_ps[:], wg[:, k, mi * 128:(mi + 1) * 128],
                               n3[k][:], k == 0, k == DC - 1)
                        for k in range(DC):
                            mm(h_ps[:], wg[:, k, gw + mi * 128:gw + (mi + 1) * 128],
                               n3[k][:], k == 0, k == DC - 1)
                        ge = gt.tile([128, S], f32, tag="ge", name="ge")
                        nc.scalar.activation(ge[:], g_ps[:], A.Gelu,
                                             bias=bias("b1")[:, mI:mI + 1],
                                             scale=ge_scale)
                        nc.vector.scalar_tensor_tensor(mT[mI][:], h_ps[:],
                                                       bias("b3")[:, mI:mI + 1],
                                                       ge[:], OP.add, OP.mult)

            # down-proj, token-major out: y[t,n] = sum_f M^T[f,t].T @ w2[f,n]
            with tc.tile_pool(name="y_ps", bufs=1, space="PSUM") as yp, \
                 tc.tile_pool(name="outp", bufs=3) as op_:
                y_ps = [[yp.tile([128, 512], f32, tag=f"y{t}{n}", name=f"y{t}{n}")
                         for n in range(2)] for t in range(QC)]
                for k2 in range(FC):
                    for t in range(QC):
                        for n in range(2):
                            mm(y_ps[t][n][:], mT[k2][:, t * 128:(t + 1) * 128],
                               w2t[:, k2, n * 512:(n + 1) * 512],
                               k2 == 0, k2 == FC - 1)
                for t in range(QC):
                    for n in range(2):
                        o = op_.tile([128, 512], f32, tag="o_out", name="o_out")
                        nc.vector.tensor_scalar_mul(o[:], y_ps[t][n][:],
                                                    bias("c")[:, 0:1])
                        nc.sync.dma_start(
                            out_ffn.ap()[t * 128:(t + 1) * 128,
                                         n * 512:(n + 1) * 512], o[:])

    nc.compile()
    return nc


class _FixedPool:
    """Adapter letting project_fm write into pre-allocated tile slices."""

    def __init__(self, tiles):
        self._tiles = list(tiles)
        self._i = 0

    def tile(self, shape, dtype, tag=None, name=None):
        t = self._tiles[self._i]
        self._i += 1
        return t


def _routing(langs):
    """Per-sequence expert slots [(expert_idx, coef) x2], matching the
    reference: coef[e,b] = any(langs[b]==4+e) * (1/count(langs[b]>3))."""
    langs = np.asarray(langs)
    slots = []
    for b in range(langs.shape[0]):
        row = [int(v) for v in langs[b]]
        cnt = sum(1 for v in row if v > 3)
        rw = 1.0 if cnt == 0 else 1.0 / cnt
        seen = []
        for v in row:
            if v > 3 and 0 <= v - 4 < NE and (v - 4) not in seen:
                seen.append(v - 4)
        sl = [(e, rw) for e in seen]
        while len(sl) < 2:
            sl.append((0, 0.0))
        slots.append(sl[:2])
    return slots


def _mask_classes(maskT, n_kc):
    """Classify each [128 keys x 128 queries] block of a transposed mask:
    0 all-zero (no add), 1 general (add), 2 fully masked (skip compute).
    Keeps at least one active key block per query and contiguous active
    ranges per key chunk."""
    cls = []
    for kc in range(n_kc):
        for qh in range(QC):
            blk = maskT[kc * 128:(kc + 1) * 128, qh * 128:(qh + 1) * 128]
            if np.all(blk == 0):
                cls.append(0)
            elif np.all(blk <= -1e8):
                cls.append(2)
            else:
                cls.append(1)
    for qh in range(QC):
        if all(cls[kc * QC + qh] == 2 for kc in range(n_kc)):
            for kc in range(n_kc):
                cls[kc * QC + qh] = 1
    for kc in range(n_kc):
        act = [q for q in range(QC) if cls[kc * QC + q] != 2]
        if not act or act != list(range(act[0], act[-1] + 1)):
            for q in range(QC):
                if cls[kc * QC + q] == 2:
                    cls[kc * QC + q] = 1
    return tuple(cls)


def kernel(**inputs):
    mode = MODE
    np_dt = ml_dtypes.bfloat16 if mode == "bf16" else np.float32
    f32 = np.float32

    inp = {k: np.asarray(v) for k, v in inputs.items()}
    x = inp["hidden_states"].astype(f32)
    enc = inp["encoder_hidden_states"].astype(f32)
    mask = inp["attention_mask"].astype(f32)
    encmask = inp["encoder_attention_mask"].astype(f32)
    g1, b1 = inp["ln1_g"].astype(f32), inp["ln1_b"].astype(f32)
    g2, b2 = inp["ln2_g"].astype(f32), inp["ln2_b"].astype(f32)
    g3, b3 = inp["ln3_g"].astype(f32), inp["ln3_b"].astype(f32)

    def dup_kv(w):
        return np.concatenate([np.tile(w[:, 64 * j:64 * (j + 1)], (1, 2))
                               for j in range(NKV)], axis=1)

    def dup_kv_b(v):
        return np.concatenate([np.tile(v[64 * j:64 * (j + 1)], 2)
                               for j in range(NKV)])

    sc = HD ** -0.5
    qw_f = g1[:, None] * inp["sa_q_w"] * sc
    qb_f = (b1 @ inp["sa_q_w"] + inp["sa_q_b"]) * sc
    kw_f = dup_kv(g1[:, None] * inp["sa_k_w"])
    kb_f = dup_kv_b(b1 @ inp["sa_k_w"] + inp["sa_k_b"])
    vw_f = g1[:, None] * inp["sa_v_w"]
    vb_f = b1 @ inp["sa_v_w"] + inp["sa_v_b"]
    q2w_f = g2[:, None] * inp["ca_q_w"] * sc
    q2b_f = (b2 @ inp["ca_q_w"] + inp["ca_q_b"]) * sc
    k2w_f = dup_kv(inp["ca_k_w"])
    k2b_f = dup_kv_b(inp["ca_k_b"])
    w1_f = inp["moe_w1"] * g3[None, :, None]
    b1_f = np.einsum("d,edf->ef", b3, inp["moe_w1"]).astype(f32)
    w3_f = inp["moe_w3"] * g3[None, :, None]
    b3_f = np.einsum("d,edf->ef", b3, inp["moe_w3"]).astype(f32)

    maskT0 = np.ascontiguousarray(mask[:, 0].transpose(0, 2, 1))     # [B,S,S]
    encmaskT0 = np.ascontiguousarray(encmask[:, 0].transpose(0, 2, 1))
    sa_cls = _mask_classes(maskT0[0], SC)
    ca_cls = _mask_classes(encmaskT0[0], KC)
    for b in range(1, B):
        if _mask_classes(maskT0[b], SC) != sa_cls or \
           _mask_classes(encmaskT0[b], KC) != ca_cls:
            sa_cls = tuple(1 for _ in range(SC * QC))
            ca_cls = tuple(1 for _ in range(KC * QC))
            break

    key = (mode, sa_cls, ca_cls)
    if key not in _CACHE:
        _CACHE[key] = _build(mode, sa_cls, ca_cls)
    nc = _CACHE[key]

    def col128(v):
        return np.asarray(v, f32).reshape(-1, 128).T

    slots = _routing(inp["langs"])
    wqkv = np.concatenate([qw_f, kw_f, vw_f], axis=1).astype(np_dt)
    wca = np.concatenate([q2w_f, k2w_f, inp["ca_v_w"]], axis=1).astype(np_dt)

    bias_common = np.zeros((128, BIAS_W), f32)
    for nm, v in [("qb", col128(qb_f)), ("kb", col128(kb_f)),
                  ("vb", np.broadcast_to(vb_f.astype(f32), (128, KVW))),
                  ("ob", col128(inp["sa_o_b"])),
                  ("q2b", col128(q2b_f)), ("k2b", col128(k2b_f)),
                  ("v2b", np.broadcast_to(inp["ca_v_b"].astype(f32), (128, KVW))),
                  ("o2b", col128(inp["ca_o_b"]))]:
        off, w = _BIAS_COLS[nm]
        bias_common[:, off:off + w] = v

    moe_fp8 = mode == "bf16" and MOE_FP8

    def moe_cast(w):
        if moe_fp8:
            return np.clip(w * FP8_SCALE, -15.5, 15.5).astype(
                ml_dtypes.float8_e3m4)
        return w.astype(np_dt)

    coef_div = FP8_SCALE * FP8_SCALE if moe_fp8 else 1.0
    b3_scale = FP8_SCALE if moe_fp8 else 1.0

    in_maps = []
    for c in range(8):
        b, j = c // 2, c % 2
        e, coef = slots[b][j]
        xT = np.ascontiguousarray(x[b].T)
        # interleave w1/w3 by group: [w1 grp g | w3 grp g] blocks of 512 cols
        gw = GRP * 128
        w13 = np.empty((D, 2 * DE), f32)
        for g in range(NGRP):
            w13[:, g * 2 * gw:g * 2 * gw + gw] = w1_f[e][:, g * gw:(g + 1) * gw]
            w13[:, g * 2 * gw + gw:(g + 1) * 2 * gw] = w3_f[e][:, g * gw:(g + 1) * gw]
        bt = bias_common.copy()
        for nm, v in [("b1", col128(b1_f[e])),
                      ("b3", col128(b3_f[e]) * b3_scale)]:
            off, w = _BIAS_COLS[nm]
            bt[:, off:off + w] = v
        bt[:, _BIAS_COLS["c"][0]] = coef / coef_div
        m = {
            "xT": xT,
            "encT": np.ascontiguousarray(enc[b].T).astype(np_dt),
            "id128": np.eye(128, dtype=f32).astype(np_dt),
            "wqkv": wqkv, "wca": wca,
            "ow": inp["sa_o_w"].astype(np_dt), "o2w": inp["ca_o_w"].astype(np_dt),
            "biases": bt,
            "w13": moe_cast(w13),
            "w2": moe_cast(np.ascontiguousarray(inp["moe_w2"][e])),
        }
        if mode != "f32":
            m["xT_st"] = xT.astype(np_dt)
        if any(cc == 1 for cc in sa_cls):
            m["maskT"] = maskT0[b].astype(np_dt)
        if any(cc == 1 for cc in ca_cls):
            m["encmaskT"] = encmaskT0[b].astype(np_dt)
        in_maps.append(m)

    kw = {}
    if _TRACE_DIR:
        kw = dict(trace=True, tmpdir=_TRACE_DIR, trace_cores=[0])
    res = bass_utils.run_bass_kernel_spmd(nc, in_maps, core_ids=list(range(8)), **kw)
    global _LAST_EXEC_NS
    _LAST_EXEC_NS = res.exec_time_ns
    return np.stack([
        res.results[2 * b]["out_res"].T
        + res.results[2 * b]["out_ffn"]
        + res.results[2 * b + 1]["out_ffn"]
        for b in range(B)
    ]).astype(f32)


# revision 31
# speedup vs baseline: 1.2860x; 1.0118x over previous
"""MBart MoE decoder layer on 8 trn2 NeuronCores.

Sharding: 8 cores = 8 (sequence, expert-slot) pairs. Core c handles
sequence b=c//2, expert slot j=c%2 (each sequence is lang-routed to at
most 2 distinct experts; routing is computed on the host from `langs`).
Each core computes the full attention path for its sequence (replicated
across the pair) and one expert FFN over all 256 tokens; the host sums
the pair's partial outputs (expert-sharded combine) and transposes back
to token-major. Expert weights are gathered per-core on the host, so a
core only receives the one expert it needs.

On-device layout is feature-major [D, tokens]: projections take weights
as lhsT (feature-major out) or activations as lhsT (token-major out), so
no activation transposes are needed anywhere. LN gains/biases are folded
into the downstream weights on the host; softmax uses transposed scores
[keys, queries] with the attention mask added via an identity-matmul
into PSUM (host classifies each 128x128 mask block as zero / add / skip,
so causal dead blocks are never computed) and denominators accumulated
via a ones-matmul, then broadcast over partitions with a rank-1 matmul;
the per-token reciprocal runs on the vector engine (reciprocal_approx_
fast) so the scalar engine keeps its exp table loaded across the whole
attend (ACT table reloads cost ~1.3us each).

DMA strategy: weights/inputs load as one packed [128, nchunk, W] tile
per tensor (single descriptor set, sync queue), while the big MoE
weights stream on the otherwise-idle gpsimd queue: w2 (fp8, resident in
SBUF from t~0) and w13 double-buffered 2MB groups. fp8 is used only for
the moving-operand w2 down-projection (quantization error there is
diluted by the residual path).
"""

import os
import sys
from contextlib import ExitStack

for _p in ("/opt/trn_rl_repo",):
    if _p not in sys.path:
        sys.path.append(_p)

import numpy as np
import ml_dtypes

import concourse.bass as bass
import concourse.tile as tile
import concourse.mybir as mybir
from concourse import bacc, bass_utils

B, S, SK = 4, 256, 512
D, NH, NKV, HD = 1024, 16, 4, 64
DE, NE = 4096, 8
LN_EPS = 1e-5
REP = NH // NKV
DC = D // 128    # 8 feature chunks
FC = DE // 128   # 32 ffn chunks
SC = S // 128    # 2 self-attn key chunks
KC = SK // 128   # 4 cross-attn key chunks
QC = S // 128    # 2 query halves
KVW = NKV * HD   # 256
GRP = 4          # ffn chunks per MoE weight group
NGRP = FC // GRP

MODE = os.environ.get("KERNEL_MM_DTYPE", "bf16")  # "bf16" | "f32r" | "f32"
MOE_FP8 = os.environ.get("KERNEL_MOE_FP8", "1") == "1"
FP8_SCALE = 128.0  # e3m4 normals start at 0.25; 0.02-scale weights need the boost

_CACHE: dict = {}
_TRACE_DIR = None   # set by test harness for profiling runs
_LAST_EXEC_NS = None

# packed attention-weight column layout: qw | kw(dup) | vw
W_Q, W_K, W_V = 0, D, D + 2 * KVW
WPACK = D + 2 * KVW + KVW  # 1792

# packed per-partition bias column layout
_BIAS_COLS = {}
_off = 0
for _n, _w in [("qb", DC), ("kb", 4), ("vb", KVW), ("ob", DC),
               ("q2b", DC), ("k2b", 4), ("v2b", KVW), ("o2b", DC),
               ("b1", FC), ("b3", FC), ("c", 1)]:
    _BIAS_COLS[_n] = (_off, _w)
    _off += _w
BIAS_W = _off


def _build(mode, sa_cls, ca_cls):
    """sa_cls/ca_cls: block classes per (kc, qhalf): 0=no-mask, 1=mask-add,
    2=fully-masked(skip)."""
    st = {"bf16": mybir.dt.bfloat16, "f32r": mybir.dt.float32r,
          "f32": mybir.dt.float32}[mode]
    f32 = mybir.dt.float32
    moe_fp8 = mode == "bf16" and MOE_FP8
    moe_dt = mybir.dt.float8e3 if moe_fp8 else st
    same_st = mode == "f32"
    A = mybir.ActivationFunctionType
    OP = mybir.AluOpType

    nc = bacc.Bacc("TRN2", target_bir_lowering=False, debug=False, num_devices=8)

    def mm(psum, lhsT, rhs, start, stop):
        nc.tensor.matmul(psum, lhsT, rhs, start=start, stop=stop)

    di = {}

    def din(name, shape, dtype=None):
        di[name] = nc.dram_tensor(name, list(shape), dtype or st, kind="ExternalInput")
        return di[name]

    # all multi-chunk tensors arrive host-packed partition-major:
    # [128, nchunk*W] with row p = concat_k src[k*128+p, :].  DMAs are then
    # plain contiguous 2D transfers (a [128, k, W] gather pattern costs
    # ~28ns/descriptor x 128*k on the HWDGE = tens of us of issue time).
    din("xT", (128, DC * S), f32)
    if not same_st:
        din("xT_st", (128, DC * S))
    din("encT", (128, DC * SK))
    need_samask = any(c == 1 for c in sa_cls)
    need_camask = any(c == 1 for c in ca_cls)
    if need_samask:
        din("maskT", (128, SC * S))
    if need_camask:
        din("encmaskT", (128, KC * S))
    din("id128", (128, 128))
    din("wqkv", (128, DC * WPACK))
    din("wca", (128, DC * WPACK))
    din("ow", (128, DC * D))
    din("o2w", (128, DC * D))
    din("biases", (128, BIAS_W), f32)
    din("w13", (128, 2 * DE * DC), moe_dt)  # [p][(g,k,c)] c=2*gw
    din("w2", (128, FC * D), moe_dt)
    out_res = nc.dram_tensor("out_res", [128, DC * S], f32,
                             kind="ExternalOutput")
    out_ffn = nc.dram_tensor("out_ffn", [S, D], f32, kind="ExternalOutput")

    def packed(dram, width):
        """[128, nchunk*W] packed dram -> [128, nchunk, W] AP."""
        return dram.ap().rearrange("p (k c) -> p k c", c=width)

    with tile.TileContext(nc) as tc, ExitStack() as ctx:
        cp = ctx.enter_context(tc.tile_pool(name="consts", bufs=1))
        pers = ctx.enter_context(tc.tile_pool(name="pers", bufs=1))

        ones128 = cp.tile([128, 1], st, tag="ones128", name="ones128")
        nc.vector.memset(ones128, 1.0)
        ones1r = cp.tile([1, 128], st, tag="ones1r", name="ones1r")
        nc.vector.memset(ones1r, 1.0)
        eps_t = cp.tile([128, 1], f32, tag="eps_t", name="eps_t")
        nc.vector.memset(eps_t, LN_EPS)
        id128 = cp.tile([128, 128], st, tag="id128", name="id128")
        nc.sync.dma_start(id128[:], di["id128"].ap())
        maskT = encmaskT = None
        if need_samask:
            maskT = cp.tile([128, SC, S], st, tag="maskT", name="maskT")
            nc.sync.dma_start(maskT[:], packed(di["maskT"], S))
        if need_camask:
            encmaskT = cp.tile([128, KC, S], st, tag="encmaskT", name="encmaskT")
            nc.sync.dma_start(encmaskT[:], packed(di["encmaskT"], S))

        bias_t = cp.tile([128, BIAS_W], f32, tag="bias_t", name="bias_t")
        nc.sync.dma_start(bias_t[:], di["biases"].ap())

        # MoE down-proj weights are SBUF-resident (fp8); the DMA issues from
        # the scalar queue after the SA exps so the transfer lands in the
        # DMA-idle SA-attend window instead of starving the critical-path
        # attention loads. w1/w3 stream as double-buffered fp8 groups.
        w2t = pers.tile([128, FC, D], moe_dt, tag="w2t", name="w2t")
        w13pool = ctx.enter_context(tc.tile_pool(name="w13p", bufs=2))
        gw = GRP * 128
        w13ap = packed(di["w13"], DC * 2 * gw).rearrange(
            "p g (k c) -> p g k c", c=2 * gw)

        def w13_group(g, engine):
            wg = w13pool.tile([128, DC, 2 * gw], moe_dt, tag="w13g",
                              name="w13g")
            engine.dma_start(wg[:], w13ap[:, g])
            return wg

        def bias(nm):
            off, w = _BIAS_COLS[nm]
            return bias_t[:, off:off + w]

        def load_packed(dram, nchunk, width, tag, pool, dtype=st, nsplit=1):
            t = pool.tile([128, nchunk, width], dtype, tag=tag, name=tag)
            ap = packed(dram, width)
            step = nchunk // nsplit
            for s in range(nsplit):
                nc.sync.dma_start(t[:, s * step:(s + 1) * step, :],
                                  ap[:, s * step:(s + 1) * step, :])
            return [t[:, k, :] for k in range(nchunk)]

        def layernorm(src_f32, src_st, out_tag, pool):
            """src: DC chunks [128,S] f32 (+st copies). Returns DC normalized
            chunks [128,S] st (gain/bias folded downstream by host)."""
            with tc.tile_pool(name=f"{out_tag}_lt", bufs=2) as lp, \
                 tc.tile_pool(name=f"{out_tag}_lp", bufs=1, space="PSUM") as sp, \
                 tc.tile_pool(name=f"{out_tag}_lb", bufs=1, space="PSUM") as bp:
                sum_ps = sp.tile([1, S], f32, tag="lnsum", name="lnsum")
                sq_ps = sp.tile([1, S], f32, tag="lnsq", name="lnsq")
                for k in range(DC):
                    sq = lp.tile([128, S], st, tag="lnsqt", name="lnsqt")
                    nc.vector.tensor_tensor(sq[:], src_f32[k][:], src_f32[k][:],
                                            OP.mult)
                    mm(sum_ps[:], ones128[:], src_st[k][:], k == 0, k == DC - 1)
                    mm(sq_ps[:], ones128[:], sq[:], k == 0, k == DC - 1)
                s_sb = lp.tile([1, S], st, tag="ln_ssb", name="ln_ssb")
                nc.vector.tensor_single_scalar(s_sb[:], sum_ps[:], 1.0 / D, OP.mult)
                q_sb = lp.tile([1, S], st, tag="ln_qsb", name="ln_qsb")
                nc.vector.tensor_single_scalar(q_sb[:], sq_ps[:], 1.0 / D, OP.mult)
                s_bc = bp.tile([128, S], f32, tag="ln_sbc", name="ln_sbc")
                q_bc = bp.tile([128, S], f32, tag="ln_qbc", name="ln_qbc")
                mm(s_bc[:], ones1r[:], s_sb[:], True, True)   # mean, bcast
                mm(q_bc[:], ones1r[:], q_sb[:], True, True)   # E[x^2], bcast
                # full-lane stats math on [128,S]
                s_sbuf = lp.tile([128, S], f32, tag="ln_ssbuf", name="ln_ssbuf")
                nc.vector.tensor_copy(s_sbuf[:], s_bc[:])
                var = lp.tile([128, S], f32, tag="ln_var", name="ln_var")
                nc.vector.scalar_tensor_tensor(var[:], s_bc[:], 0.0, s_sbuf[:],
                                               OP.bypass, OP.mult)
                nc.vector.tensor_sub(var[:], q_bc[:], var[:])
                v_t = lp.tile([128, S], f32, tag="ln_vt", name="ln_vt")
                nc.scalar.activation(v_t[:], var[:], A.Abs_reciprocal_sqrt,
                                     bias=eps_t[:])
                u_t = lp.tile([128, S], f32, tag="ln_ut", name="ln_ut")
                nc.vector.tensor_tensor(u_t[:], s_sbuf[:], v_t[:], OP.mult)
                outs = []
                for k in range(DC):
                    o = pool.tile([128, S], st, tag=f"{out_tag}{k}",
                                  name=f"{out_tag}{k}")
                    nc.vector.tensor_tensor(o[:], src_f32[k][:], v_t[:], OP.mult)
                    nc.vector.tensor_sub(o[:], o[:], u_t[:])
                    outs.append(o)
                return outs

        def cast_st(src, tag, pool):
            if same_st:
                return src
            outs = []
            for k, t in enumerate(src):
                o = pool.tile([128, S], st, tag=f"{tag}{k}",
                              name=f"{tag}{k}")
                nc.vector.tensor_copy(o[:], t[:])
                outs.append(o)
            return outs

        def project_fm(w_slices, rhs_chunks, nout, bias_ap, out_tag, pool,
                       extra=None, out_dt=None, width=None):
            """out^T[dout_chunk] = sum_k w_slices[k][:, m*128:...].T @ rhs[k]."""
            W = width or S
            outs = []
            with tc.tile_pool(name=f"{out_tag}_ps", bufs=3, space="PSUM") as pp:
                for mI in range(nout):
                    ps = pp.tile([128, W], f32, tag="proj", name="proj")
                    for k in range(DC):
                        mm(ps[:], w_slices[k][:, mI * 128:(mI + 1) * 128],
                           rhs_chunks[k][:], k == 0, k == DC - 1)
                    o = pool.tile([128, W], out_dt or st, tag=f"{out_tag}{mI}",
                                  name=f"{out_tag}{mI}")
                    if extra is not None:
                        extra(mI, ps, o)
                    elif bias_ap is not None:
                        nc.vector.tensor_scalar(o[:], ps[:],
                                                bias_ap[:, mI:mI + 1], None,
                                                OP.add)
                    else:
                        nc.vector.tensor_copy(o[:], ps[:])
                    outs.append(o)
            return outs

        def project_tm(act_chunks, w_slices, ntok, bias_bcast, out_tag, pool):
            """token-major V with a ones column appended per kv head:
            out[tok_chunk] = [V_kv | 1] blocks of 65 columns."""
            outs = []
            with tc.tile_pool(name=f"{out_tag}_ps", bufs=3, space="PSUM") as pp:
                for t in range(ntok):
                    ps = pp.tile([128, KVW], f32, tag="projtm", name="projtm")
                    for k in range(DC):
                        mm(ps[:], act_chunks[k][:, t * 128:(t + 1) * 128],
                           w_slices[k][:], k == 0, k == DC - 1)
                    o = pool.tile([128, NKV, HD + 1], st, tag=f"{out_tag}{t}",
                                  name=f"{out_tag}{t}")
                    nc.vector.tensor_add(
                        o[:, :, 0:HD],
                        ps[:].rearrange("p (kv d) -> p kv d", kv=NKV),
                        bias_bcast[:].rearrange("p (kv d) -> p kv d", kv=NKV))
                    for kv in range(NKV):
                        nc.vector.tensor_copy(o[:, kv, HD:HD + 1], ones128[:])
                    outs.append(o)
            return outs

        def attend_v2(qT, kT, vtm, n_kc, mask_tile, cls, out_tag, pool):
            """rev2-style attend (separate per-head tiles) for bisection."""
            outs = []
            qr = []
            for kc in range(n_kc):
                act = [qh for qh in range(QC) if cls[kc * QC + qh] != 2]
                assert act and act == list(range(act[0], act[-1] + 1))
                qr.append((act[0] * 128, (act[-1] + 1) * 128))
            with tc.tile_pool(name=f"{out_tag}_sp", bufs=3, space="PSUM") as stp, \
                 tc.tile_pool(name=f"{out_tag}_op", bufs=2, space="PSUM") as opp, \
                 tc.tile_pool(name=f"{out_tag}_bp", bufs=1, space="PSUM") as bpp, \
                 tc.tile_pool(name=f"{out_tag}_et", bufs=6) as epool, \
                 tc.tile_pool(name=f"{out_tag}_dt", bufs=3) as dpool:
                for c in range(DC):
                    o_ps_h = [opp.tile([65, S], f32, tag=f"oph{hh}",
                                       name=f"oph{hh}") for hh in range(2)]
                    kv = (2 * c) // REP
                    for kc in range(n_kc):
                        q0, q1 = qr[kc]
                        adds = [q for q in range(QC) if cls[kc * QC + q] == 1]
                        st_h = []
                        e_h = []
                        for hh in range(2):
                            qh_ap = qT[c][hh * 64:(hh + 1) * 64, :]
                            kh = kT[kv][hh * 64:(hh + 1) * 64, :]
                            st_ps = stp.tile([128, S], f32, tag="st",
                                             name="st")
                            mm(st_ps[:, q0:q1], kh[:, kc * 128:(kc + 1) * 128],
                               qh_ap[:, q0:q1], True, not adds)
                            st_h.append(st_ps)
                        for hh in range(2):
                            for i, q in enumerate(adds):
                                mm(st_h[hh][:, q * 128:(q + 1) * 128], id128[:],
                                   mask_tile[:, kc, q * 128:(q + 1) * 128],
                                   False, i == len(adds) - 1)
                        for hh in range(2):
                            e = epool.tile([128, S], st, tag="e", name="e")
                            nc.scalar.activation(e[:, q0:q1],
                                                 st_h[hh][:, q0:q1], A.Exp)
                            e_h.append(e)
                        for hh in range(2):
                            mm(o_ps_h[hh][:, q0:q1],
                               vtm[kc][:, kv, :], e_h[hh][:, q0:q1],
                               kc == 0, kc == n_kc - 1)
                    den_pair = dpool.tile([1, 2 * S], st, tag="den_pair",
                                          name="den_pair")
                    for hh in range(2):
                        nc.vector.tensor_copy(den_pair[:, hh * S:(hh + 1) * S],
                                              o_ps_h[hh][64:65, :])
                    r_ps = bpp.tile([128, 2 * S], f32, tag="rbc", name="rbc")
                    mm(r_ps[:], ones1r[:], den_pair[:], True, True)
                    rbi = dpool.tile([128, 2 * S], f32, tag="rbi", name="rbi")
                    nc.vector.reciprocal_approx_fast(rbi[:], r_ps[:])
                    o = pool.tile([128, S], st, tag=f"{out_tag}{c}",
                                  name=f"{out_tag}{c}")
                    for hh in range(2):
                        nc.vector.tensor_tensor(
                            o[hh * 64:(hh + 1) * 64, :], o_ps_h[hh][0:64, :],
                            rbi[hh * 64:(hh + 1) * 64, hh * S:(hh + 1) * S],
                            OP.mult)
                    outs.append(o)
            return outs

        def attend(qT, kT, vtm, n_kc, mask_tile, cls, out_tag, pool):
            v = os.environ.get("KERNEL_ATT", "2")
            fn = {"2": attend_v2, "1": attend_h1, "3": attend_v3}[v]
            return fn(qT, kT, vtm, n_kc, mask_tile, cls, out_tag, pool)

        def attend_h1(qT, kT, vtm, n_kc, mask_tile, cls, out_tag, pool):
            """Hybrid: st_pair bank + fused exp, but rev2-style separate
            o_ps_h tiles and per-head AV."""
            outs = []
            qr = []
            for kc in range(n_kc):
                act = [qh for qh in range(QC) if cls[kc * QC + qh] != 2]
                assert act and act == list(range(act[0], act[-1] + 1))
                qr.append((act[0] * 128, (act[-1] + 1) * 128))
            with tc.tile_pool(name=f"{out_tag}_sp", bufs=3, space="PSUM") as stp, \
                 tc.tile_pool(name=f"{out_tag}_op", bufs=2, space="PSUM") as opp, \
                 tc.tile_pool(name=f"{out_tag}_bp", bufs=1, space="PSUM") as bpp, \
                 tc.tile_pool(name=f"{out_tag}_et", bufs=6) as epool, \
                 tc.tile_pool(name=f"{out_tag}_dt", bufs=3) as dpool:
                for c in range(DC):
                    o_ps_h = [opp.tile([65, S], f32, tag=f"oph{hh}",
                                       name=f"oph{hh}") for hh in range(2)]
                    kv = (2 * c) // REP
                    for kc in range(n_kc):
                        q0, q1 = qr[kc]
                        full = (q0, q1) == (0, S)
                        adds = [q for q in range(QC) if cls[kc * QC + q] == 1]
                        st_pair = stp.tile([128, 2, S], f32, tag="st",
                                           name="st")
                        for hh in range(2):
                            qh_ap = qT[c][hh * 64:(hh + 1) * 64, :]
                            kh = kT[kv][hh * 64:(hh + 1) * 64, :]
                            mm(st_pair[:, hh, q0:q1],
                               kh[:, kc * 128:(kc + 1) * 128],
                               qh_ap[:, q0:q1], True, not adds)
                            for i, q in enumerate(adds):
                                mm(st_pair[:, hh, q * 128:(q + 1) * 128],
                                   id128[:],
                                   mask_tile[:, kc, q * 128:(q + 1) * 128],
                                   False, i == len(adds) - 1)
                        e = epool.tile([128, 2, S], st, tag="e", name="e")
                        if full:
                            nc.scalar.activation(e[:, :, :], st_pair[:, :, :],
                                                 A.Exp)
                        else:
                            for hh in range(2):
                                nc.scalar.activation(e[:, hh, q0:q1],
                                                     st_pair[:, hh, q0:q1],
                                                     A.Exp)
                        for hh in range(2):
                            mm(o_ps_h[hh][:, q0:q1], vtm[kc][:, kv, :],
                               e[:, hh, q0:q1], kc == 0, kc == n_kc - 1)
                    den_pair = dpool.tile([1, 2 * S], st, tag="den_pair",
                                          name="den_pair")
                    for hh in range(2):
                        nc.vector.tensor_copy(den_pair[:, hh * S:(hh + 1) * S],
                                              o_ps_h[hh][64:65, :])
                    r_ps = bpp.tile([128, 2 * S], f32, tag="rbc", name="rbc")
                    mm(r_ps[:], ones1r[:], den_pair[:], True, True)
                    rbi = dpool.tile([128, 2 * S], f32, tag="rbi", name="rbi")
                    nc.vector.reciprocal_approx_fast(rbi[:], r_ps[:])
                    o = pool.tile([128, S], st, tag=f"{out_tag}{c}",
                                  name=f"{out_tag}{c}")
                    for hh in range(2):
                        nc.vector.tensor_tensor(
                            o[hh * 64:(hh + 1) * 64, :], o_ps_h[hh][0:64, :],
                            rbi[hh * 64:(hh + 1) * 64, hh * S:(hh + 1) * S],
                            OP.mult)
                    outs.append(o)
            return outs

        def attend_v3(qT, kT, vtm, n_kc, mask_tile, cls, out_tag, pool):
            """Transposed-score attention, head-PAIR fused. cls[kc*QC + qh]
            in {0,1,2}. Scores for both heads of a pair land in the two
            halves of ONE PSUM bank (sequential accumulation groups: start=
            True clears the whole bank's has_written bits, so group hh=1 only
            begins after group hh=0's stop). The exp and the O' matmul then
            process both heads in one instruction; vtm blocks are [V_kv | 1]
            of 65 cols shared by the pair, so O' also accumulates the softmax
            denominators into row 64."""
            outs = []
            # per kc: active query range (contiguous union of non-skip halves)
            qr = []
            for kc in range(n_kc):
                act = [qh for qh in range(QC) if cls[kc * QC + qh] != 2]
                assert act and act == list(range(act[0], act[-1] + 1))
                qr.append((act[0] * 128, (act[-1] + 1) * 128))
            with tc.tile_pool(name=f"{out_tag}_sp", bufs=3, space="PSUM") as stp, \
                 tc.tile_pool(name=f"{out_tag}_op", bufs=2, space="PSUM") as opp, \
                 tc.tile_pool(name=f"{out_tag}_bp", bufs=2, space="PSUM") as bpp, \
                 tc.tile_pool(name=f"{out_tag}_et", bufs=6) as epool, \
                 tc.tile_pool(name=f"{out_tag}_dt", bufs=3) as dpool:
                for c in range(DC):
                    o_pair = opp.tile([65, 2, S], f32, tag="opair",
                                      name="opair")
                    kv = (2 * c) // REP      # same kv head for both of the pair
                    for kc in range(n_kc):
                        q0, q1 = qr[kc]
                        full = (q0, q1) == (0, S)
                        adds = [q for q in range(QC) if cls[kc * QC + q] == 1]
                        st_pair = stp.tile([128, 2, S], f32, tag="st",
                                           name="st")
                        # ONE accumulation group covers both heads' halves of
                        # the bank: start=True only on the very first matmul
                        # (a second start would clear the whole bank's
                        # has_written bits mid-flight -> device hang).
                        n_mm = 2 * (1 + len(adds))
                        i_mm = 0
                        for hh in range(2):
                            qh_ap = qT[c][hh * 64:(hh + 1) * 64, :]
                            kh = kT[kv][hh * 64:(hh + 1) * 64, :]
                            mm(st_pair[:, hh, q0:q1],
                               kh[:, kc * 128:(kc + 1) * 128],
                               qh_ap[:, q0:q1], i_mm == 0, i_mm == n_mm - 1)
                            i_mm += 1
                            for q in adds:
                                mm(st_pair[:, hh, q * 128:(q + 1) * 128],
                                   id128[:],
                                   mask_tile[:, kc, q * 128:(q + 1) * 128],
                                   False, i_mm == n_mm - 1)
                                i_mm += 1
                        e = epool.tile([128, 2, S], st, tag="e", name="e")
                        if full and os.environ.get("KERNEL_NOFUSE") != "1":
                            # contiguous pair: one exp + one O' matmul
                            nc.scalar.activation(e[:, :, :], st_pair[:, :, :],
                                                 A.Exp)
                            mm(o_pair[:, :, :], vtm[kc][:, kv, :], e[:, :, :],
                               kc == 0, kc == n_kc - 1)
                        else:
                            # partial q-range: strided free-dim APs are not
                            # supported by ACT/PE -> per-head contiguous ops
                            for hh in range(2):
                                nc.scalar.activation(e[:, hh, q0:q1],
                                                     st_pair[:, hh, q0:q1],
                                                     A.Exp)
                            for hh in range(2):
                                mm(o_pair[:, hh, q0:q1], vtm[kc][:, kv, :],
                                   e[:, hh, q0:q1],
                                   kc == 0, kc == n_kc - 1 and hh == 1)
                    den_pair = dpool.tile([1, 2 * S], st, tag="den_pair",
                                          name="den_pair")
                    nc.vector.tensor_copy(
                        den_pair[:],
                        o_pair[64:65, :, :].rearrange("p a b -> p (a b)"))
                    r_ps = bpp.tile([128, 2 * S], f32, tag="rbc", name="rbc")
                    mm(r_ps[:], ones1r[:], den_pair[:], True, True)
                    # 1/x on the vector engine: keeps ACT's exp table loaded
                    # (an exp<->rsqrt table swap costs ~1.3us each way)
                    rbi = dpool.tile([128, 2 * S], f32, tag="rbi", name="rbi")
                    nc.vector.reciprocal_approx_fast(rbi[:], r_ps[:])
                    o = pool.tile([128, S], st, tag=f"{out_tag}{c}",
                                  name=f"{out_tag}{c}")
                    for hh in range(2):
                        nc.vector.tensor_tensor(
                            o[hh * 64:(hh + 1) * 64, :], o_pair[0:64, hh, :],
                            rbi[hh * 64:(hh + 1) * 64, hh * S:(hh + 1) * S],
                            OP.mult)
                    outs.append(o)
            return outs

        h1t = pers.tile([128, DC, S], f32, tag="h1T", name="h1T")
        h2t = pers.tile([128, DC, S], f32, tag="h2T", name="h2T")
        h1 = [h1t[:, k, :] for k in range(DC)]
        h2 = [h2t[:, k, :] for k in range(DC)]

        cain = ctx.enter_context(tc.tile_pool(name="ca_in", bufs=1))

        # ---------------- self attention ----------------
        with tc.tile_pool(name="sa_acts", bufs=1) as sa:
            xT = load_packed(di["xT"], DC, S, "xT", sa, f32, nsplit=2)
            xT_st = xT if same_st else load_packed(di["xT_st"], DC, S, "xTs", sa)
            with tc.tile_pool(name="wqkvp", bufs=1) as wp:
                wt = load_packed(di["wqkv"], DC, WPACK, "wqkv", wp, nsplit=2)
                ow_t = load_packed(di["ow"], DC, D, "ow", wp)
                # cross-attn inputs prefetch behind the SA-critical loads
                encT = load_packed(di["encT"], DC, SK, "encT", cain)
                wt2 = load_packed(di["wca"], DC, WPACK, "wca", cain, nsplit=2)
                n1 = layernorm(xT, xT_st, "n1T", sa)
                qT = project_fm([t[:, W_Q:W_Q + D] for t in wt], n1, DC,
                                bias("qb"), "qT", sa)
                kT = project_fm([t[:, W_K:W_K + 2 * KVW] for t in wt], n1, 4,
                                bias("kb"), "kT", sa)
                v_tm = project_tm(n1, [t[:, W_V:W_V + KVW] for t in wt], SC,
                                  bias("vb"), "v_tm", sa)
                sa_out = attend(qT, kT, v_tm, SC, maskT, sa_cls, "saT", sa)
                # resident/prefetched MoE weights: issue on the scalar queue
                # here so the transfers land in the SA-attend DMA lull
                _bulk = nc.sync if os.environ.get("KERNEL_NOBULK") == "1" \
                    else nc.scalar
                w13g01 = [w13_group(0, _bulk), w13_group(1, _bulk)]
                _bulk.dma_start(w2t[:], packed(di["w2"], D))

                def o_epil(mI, ps, o):
                    nc.vector.scalar_tensor_tensor(o[:], ps[:],
                                                   bias("ob")[:, mI:mI + 1],
                                                   xT[mI][:], OP.add, OP.add)
                project_fm(ow_t, sa_out, DC, None, "h1w", _FixedPool(h1),
                           extra=o_epil, out_dt=f32)

        # ---------------- cross attention ----------------
        with tc.tile_pool(name="ca_acts", bufs=1) as ca:
            h1_st = cast_st(h1, "h1s", ca)
            with tc.tile_pool(name="wcap", bufs=1) as wp:
                wt = wt2
                k2T = project_fm([t[:, W_K:W_K + 2 * KVW] for t in wt], encT, 4,
                                 bias("k2b"), "k2T", ca, width=SK)
                v2_tm = project_tm(encT, [t[:, W_V:W_V + KVW] for t in wt], KC,
                                   bias("v2b"), "v2_tm", ca)
                n2 = layernorm(h1, h1_st, "n2T", ca)
                q2T = project_fm([t[:, W_Q:W_Q + D] for t in wt], n2, DC,
                                 bias("q2b"), "q2T", ca)
            with tc.tile_pool(name="wo2p", bufs=1) as wp:
                o2w_t = load_packed(di["o2w"], DC, D, "o2w", wp)
                ca_out = attend(q2T, k2T, v2_tm, KC, encmaskT, ca_cls, "caT", ca)

                def o2_epil(mI, ps, o):
                    nc.vector.scalar_tensor_tensor(o[:], ps[:],
                                                   bias("o2b")[:, mI:mI + 1],
                                                   h1[mI][:], OP.add, OP.add)
                project_fm(o2w_t, ca_out, DC, None, "h2w", _FixedPool(h2),
                           extra=o2_epil, out_dt=f32)

        # residual output (host: out_b = res.T + ffn_j0 + ffn_j1)
        nc.sync.dma_start(out_res.ap(),
                          h2t[:].rearrange("p k c -> p (k c)"))

        # ---------------- MoE expert ----------------
        with tc.tile_pool(name="moe_acts", bufs=1) as mo:
            h2_st = cast_st(h2, "h2s", mo)
            n3 = layernorm(h2, h2_st, "n3T", mo)

            mT = [mo.tile([128, S], st, tag=f"mT{m}", name=f"mT{m}")
                  for m in range(FC)]
            ge_scale = 1.0 / FP8_SCALE if moe_fp8 else 1.0
            with tc.tile_pool(name="gh_ps", bufs=3, space="PSUM") as gp, \
                 tc.tile_pool(name="gelu_t", bufs=3) as gt:
                for g in range(NGRP):
                    wg = w13g01[g] if g < 2 else w13_group(g, nc.gpsimd)
                    for mi in range(GRP):
                        mI = g * GRP + mi
                        g_ps = gp.tile([128, S], f32, tag="g_ps", name="g_ps")
                        h_ps = gp.tile([128, S], f32, tag="h_ps", name="h_ps")
                        for k in range(DC):
                            mm(g_ps[:], wg[:, k, mi * 128:(mi + 1) * 128],
                               n3[k][:], k == 0, k == DC - 1)
                        for k in range(DC):
                            mm(h_ps[:], wg[:, k, gw + mi * 128:gw + (mi + 1) * 128],
                               n3[k][:], k == 0, k == DC - 1)
                        ge = gt.tile([128, S], f32, tag="ge", name="ge")
                        nc.scalar.activation(ge[:], g_ps[:], A.Gelu,
                                             bias=bias("b1")[:, mI:mI + 1],
                                             scale=ge_scale)
                        nc.vector.scalar_tensor_tensor(mT[mI][:], h_ps[:],
                                                       bias("b3")[:, mI:mI + 1],
                                                       ge[:], OP.add, OP.mult)

            # down-proj, token-major out: y[t,n] = sum_f M^T[f,t].T @ w2[f,n]
            with tc.tile_pool(name="y_ps", bufs=1, space="PSUM") as yp, \
                 tc.tile_pool(name="outp", bufs=3) as op_:
                y_ps = [[yp.tile([128, 512], f32, tag=f"y{t}{n}", name=f"y{t}{n}")
                         for n in range(2)] for t in range(QC)]
                for k2 in range(FC):
                    for t in range(QC):
                        for n in range(2):
                            mm(y_ps[t][n][:], mT[k2][:, t * 128:(t + 1) * 128],
                               w2t[:, k2, n * 512:(n + 1) * 512],
                               k2 == 0, k2 == FC - 1)
                for t in range(QC):
                    for n in range(2):
                        o = op_.tile([128, 512], f32, tag="o_out", name="o_out")
                        nc.vector.tensor_scalar_mul(o[:], y_ps[t][n][:],
                                                    bias("c")[:, 0:1])
                        nc.sync.dma_start(
                            out_ffn.ap()[t * 128:(t + 1) * 128,
                                         n * 512:(n + 1) * 512], o[:])

    nc.compile()
    return nc


class _FixedPool:
    """Adapter letting project_fm write into pre-allocated tile slices."""

    def __init__(self, tiles):
        self._tiles = list(tiles)
        self._i = 0

    def tile(self, shape, dtype, tag=None, name=None):
        t = self._tiles[self._i]
        self._i += 1
        return t


def _routing(langs):
    """Per-sequence expert slots [(expert_idx, coef) x2], matching the
    reference: coef[e,b] = any(langs[b]==4+e) * (1/count(langs[b]>3))."""
    langs = np.asarray(langs)
    slots = []
    for b in range(langs.shape[0]):
        row = [int(v) for v in langs[b]]
        cnt = sum(1 for v in row if v > 3)
        rw = 1.0 if cnt == 0 else 1.0 / cnt
        seen = []
        for v in row:
            if v > 3 and 0 <= v - 4 < NE and (v - 4) not in seen:
                seen.append(v - 4)
        sl = [(e, rw) for e in seen]
        while len(sl) < 2:
            sl.append((0, 0.0))
        slots.append(sl[:2])
    return slots


def _mask_classes(maskT, n_kc):
    """Classify each [128 keys x 128 queries] block of a transposed mask:
    0 all-zero (no add), 1 general (add), 2 fully masked (skip compute).
    Keeps at least one active key block per query and contiguous active
    ranges per key chunk."""
    cls = []
    for kc in range(n_kc):
        for qh in range(QC):
            blk = maskT[kc * 128:(kc + 1) * 128, qh * 128:(qh + 1) * 128]
            if np.all(blk == 0):
                cls.append(0)
            elif np.all(blk <= -1e8):
                cls.append(2)
            else:
                cls.append(1)
    for qh in range(QC):
        if all(cls[kc * QC + qh] == 2 for kc in range(n_kc)):
            for kc in range(n_kc):
                cls[kc * QC + qh] = 1
    for kc in range(n_kc):
        act = [q for q in range(QC) if cls[kc * QC + q] != 2]
        if not act or act != list(range(act[0], act[-1] + 1)):
            for q in range(QC):
                if cls[kc * QC + q] == 2:
                    cls[kc * QC + q] = 1
    return tuple(cls)


def kernel(**inputs):
    mode = MODE
    np_dt = ml_dtypes.bfloat16 if mode == "bf16" else np.float32
    f32 = np.float32

    inp = {k: np.asarray(v) for k, v in inputs.items()}
    x = inp["hidden_states"].astype(f32)
    enc = inp["encoder_hidden_states"].astype(f32)
    mask = inp["attention_mask"].astype(f32)
    encmask = inp["encoder_attention_mask"].astype(f32)
    g1, b1 = inp["ln1_g"].astype(f32), inp["ln1_b"].astype(f32)
    g2, b2 = inp["ln2_g"].astype(f32), inp["ln2_b"].astype(f32)
    g3, b3 = inp["ln3_g"].astype(f32), inp["ln3_b"].astype(f32)

    def dup_kv(w):
        return np.concatenate([np.tile(w[:, 64 * j:64 * (j + 1)], (1, 2))
                               for j in range(NKV)], axis=1)

    def dup_kv_b(v):
        return np.concatenate([np.tile(v[64 * j:64 * (j + 1)], 2)
                               for j in range(NKV)])

    sc = HD ** -0.5
    qw_f = g1[:, None] * inp["sa_q_w"] * sc
    qb_f = (b1 @ inp["sa_q_w"] + inp["sa_q_b"]) * sc
    kw_f = dup_kv(g1[:, None] * inp["sa_k_w"])
    kb_f = dup_kv_b(b1 @ inp["sa_k_w"] + inp["sa_k_b"])
    vw_f = g1[:, None] * inp["sa_v_w"]
    vb_f = b1 @ inp["sa_v_w"] + inp["sa_v_b"]
    q2w_f = g2[:, None] * inp["ca_q_w"] * sc
    q2b_f = (b2 @ inp["ca_q_w"] + inp["ca_q_b"]) * sc
    k2w_f = dup_kv(inp["ca_k_w"])
    k2b_f = dup_kv_b(inp["ca_k_b"])
    w1_f = inp["moe_w1"] * g3[None, :, None]
    b1_f = np.einsum("d,edf->ef", b3, inp["moe_w1"]).astype(f32)
    w3_f = inp["moe_w3"] * g3[None, :, None]
    b3_f = np.einsum("d,edf->ef", b3, inp["moe_w3"]).astype(f32)

    maskT0 = np.ascontiguousarray(mask[:, 0].transpose(0, 2, 1))     # [B,S,S]
    encmaskT0 = np.ascontiguousarray(encmask[:, 0].transpose(0, 2, 1))
    sa_cls = _mask_classes(maskT0[0], SC)
    ca_cls = _mask_classes(encmaskT0[0], KC)
    for b in range(1, B):
        if _mask_classes(maskT0[b], SC) != sa_cls or \
           _mask_classes(encmaskT0[b], KC) != ca_cls:
            sa_cls = tuple(1 for _ in range(SC * QC))
            ca_cls = tuple(1 for _ in range(KC * QC))
            break

    key = (mode, sa_cls, ca_cls)
    if key not in _CACHE:
        _CACHE[key] = _build(mode, sa_cls, ca_cls)
    nc = _CACHE[key]

    def col128(v):
        return np.asarray(v, f32).reshape(-1, 128).T

    def pk(a):
        """[nchunk*128, W] -> partition-major [128, nchunk*W]."""
        a = np.asarray(a)
        n = a.shape[0] // 128
        return np.ascontiguousarray(
            a.reshape(n, 128, a.shape[1]).transpose(1, 0, 2).reshape(128, -1))

    slots = _routing(inp["langs"])
    wqkv = np.concatenate([qw_f, kw_f, vw_f], axis=1).astype(np_dt)
    wca = np.concatenate([q2w_f, k2w_f, inp["ca_v_w"]], axis=1).astype(np_dt)

    bias_common = np.zeros((128, BIAS_W), f32)
    for nm, v in [("qb", col128(qb_f)), ("kb", col128(kb_f)),
                  ("vb", np.broadcast_to(vb_f.astype(f32), (128, KVW))),
                  ("ob", col128(inp["sa_o_b"])),
                  ("q2b", col128(q2b_f)), ("k2b", col128(k2b_f)),
                  ("v2b", np.broadcast_to(inp["ca_v_b"].astype(f32), (128, KVW))),
                  ("o2b", col128(inp["ca_o_b"]))]:
        off, w = _BIAS_COLS[nm]
        bias_common[:, off:off + w] = v

    moe_fp8 = mode == "bf16" and MOE_FP8

    def moe_cast(w):
        if moe_fp8:
            return np.clip(w * FP8_SCALE, -15.5, 15.5).astype(
                ml_dtypes.float8_e3m4)
        return w.astype(np_dt)

    coef_div = FP8_SCALE * FP8_SCALE if moe_fp8 else 1.0
    b3_scale = FP8_SCALE if moe_fp8 else 1.0

    in_maps = []
    for c in range(8):
        b, j = c // 2, c % 2
        e, coef = slots[b][j]
        xT = np.ascontiguousarray(x[b].T)
        # interleave w1/w3 by group: [w1 grp g | w3 grp g] blocks of 512 cols
        gw = GRP * 128
        w13 = np.empty((D, 2 * DE), f32)
        for g in range(NGRP):
            w13[:, g * 2 * gw:g * 2 * gw + gw] = w1_f[e][:, g * gw:(g + 1) * gw]
            w13[:, g * 2 * gw + gw:(g + 1) * 2 * gw] = w3_f[e][:, g * gw:(g + 1) * gw]
        bt = bias_common.copy()
        for nm, v in [("b1", col128(b1_f[e])),
                      ("b3", col128(b3_f[e]) * b3_scale)]:
            off, w = _BIAS_COLS[nm]
            bt[:, off:off + w] = v
        bt[:, _BIAS_COLS["c"][0]] = coef / coef_div
        # w13 packed [p][(g, k, c)]: per group g, per k-chunk, 2*gw cols
        w13p = np.ascontiguousarray(
            w13.reshape(DC, 128, NGRP, 2 * gw)
               .transpose(1, 2, 0, 3).reshape(128, -1))
        m = {
            "xT": pk(xT),
            "encT": pk(enc[b].T).astype(np_dt),
            "id128": np.eye(128, dtype=f32).astype(np_dt),
            "wqkv": pk(wqkv), "wca": pk(wca),
            "ow": pk(inp["sa_o_w"].astype(np_dt)),
            "o2w": pk(inp["ca_o_w"].astype(np_dt)),
            "biases": bt,
            "w13": moe_cast(w13p),
            "w2": pk(moe_cast(np.ascontiguousarray(inp["moe_w2"][e]))),
        }
        if mode != "f32":
            m["xT_st"] = pk(xT).astype(np_dt)
        if any(cc == 1 for cc in sa_cls):
            m["maskT"] = pk(maskT0[b]).astype(np_dt)
        if any(cc == 1 for cc in ca_cls):
            m["encmaskT"] = pk(encmaskT0[b]).astype(np_dt)
        in_maps.append(m)

    kw = {}
    if _TRACE_DIR:
        kw = dict(trace=True, tmpdir=_TRACE_DIR, trace_cores=[0])
    res = bass_utils.run_bass_kernel_spmd(nc, in_maps, core_ids=list(range(8)), **kw)
    global _LAST_EXEC_NS
    _LAST_EXEC_NS = res.exec_time_ns
    def unpk_res(a):
        """packed [128, DC*S] -> [S, D]"""
        return a.reshape(128, DC, S).transpose(1, 0, 2).reshape(D, S).T

    return np.stack([
        unpk_res(res.results[2 * b]["out_res"])
        + res.results[2 * b]["out_ffn"]
        + res.results[2 * b + 1]["out_ffn"]
        for b in range(B)
    ]).astype(f32)
